# revision 17
# baseline (speedup 1.0000x reference)
"""Cross-modal attention block on 8 Trainium2 NeuronCores.

Sharding: core = 2*b + g  ->  batch b (4-way data parallel) x head-group g
(2-way tensor parallel over 16 heads -> 8 heads/core).  Each core:
  rownorm(x[b]) -> PE transpose -> q projection (ternary weights, gamma/beta
  folded) ; kT/v projections from pre-transposed context ; per-head
  scoresT = k~^T q~ ; exp split between ScalarE (exact) and VectorE
  (Schraudolph bit-trick into bf16) ; unnormalized attn-out with an appended
  ones-row producing softmax denominators in the same matmul ; deferred
  batch normalize (reciprocal_approx_fast + select-matmul broadcast) ;
  out-proj partial.  Host sums the two partials per batch + residual +
  folded biases.

All matmuls are full 128x128-mode bf16 (scores use zero-padded K so the PE
never enters a tiled mode, which measures as HAM-throttled 1.2 GHz).
"""

import os

import ml_dtypes
import numpy as np

import concourse.bass as bass
import concourse.mybir as mybir
import concourse.tile as tile
from concourse import bacc
from concourse.bass_utils import run_bass_kernel_spmd
from concourse.masks import make_identity

FP = mybir.dt.float32
FPR = mybir.dt.float32r
BF = mybir.dt.bfloat16
I16 = mybir.dt.int16
F8 = mybir.dt.float8e4

B, T, TC, C = 4, 1024, 2048, 1024
H, HD = 16, 64
HL = 8           # heads per core
CL = HL * HD     # 512 local channels
SCALE = HD ** -0.5
LN_EPS = 1e-5
Q_EPS = 1e-5
P = 128
NCORES = 8

NT = T // P      # 8 query-row tiles
NKC = C // P     # 8 contraction chunks over C
NJ = TC // P     # 16 context chunks
NM = CL // P     # 4 local d-chunks

# Schraudolph fast-exp into bf16 bit pattern via int16:
#   i16 = trunc(x * EXP_A + EXP_B); bf16 = bits(i16)
# max rel err ~3.3% over x in [-10, 8]; scores*scale stay well inside.
EXP_A = float(np.float32(128.0 / np.log(2.0)))
EXP_B = float(np.float32(16256.0 - 5.1))

last_exec_time_ns = None


def _build_nc():
    nc = bacc.Bacc(None, target_bir_lowering=False, debug=False)

    x_d = nc.dram_tensor("x", [NT, P, C], BF, kind="ExternalInput")
    ctxT_d = nc.dram_tensor("ctxT", [P, NKC, TC], F8, kind="ExternalInput")
    wqT_d = nc.dram_tensor("wqT", [P, NKC, CL], BF, kind="ExternalInput")
    wkT_d = nc.dram_tensor("wkT", [P, NKC, CL], F8, kind="ExternalInput")
    wvT_d = nc.dram_tensor("wvT", [P, NKC, CL], F8, kind="ExternalInput")
    woT_d = nc.dram_tensor("woT", [P, NM, C], BF, kind="ExternalInput")
    cb_d = nc.dram_tensor("cb", [P, 9], FP, kind="ExternalInput")
    sel_d = nc.dram_tensor("sel", [P, NM, P], BF, kind="ExternalInput")
    part_d = nc.dram_tensor("partial", [C // P, P, T], BF, kind="ExternalOutput")

    with tile.TileContext(nc) as tc:
        with (
            tc.tile_pool(name="const", bufs=1) as cpool,
            tc.tile_pool(name="acts", bufs=1) as apool,
        ):
            ident_f = cpool.tile([P, P], FP)
            make_identity(nc, ident_f[:])
            ident = cpool.tile([P, P], BF)
            nc.vector.tensor_copy(ident[:], ident_f[:])
            cb = cpool.tile([P, 9], FP)
            nc.sync.dma_start(cb[:], cb_d[:])
            sel = cpool.tile([P, NM, P], BF)
            nc.sync.dma_start(sel[:], sel_d[:])
            eps = cpool.tile([P, 1], FP)
            nc.vector.memset(eps[:], LN_EPS)

            wk = apool.tile([P, NKC, CL], F8, tag="wk")
            wv = apool.tile([P, NKC, CL], F8, tag="wv")
            wo = apool.tile([P, NM, C], BF, tag="wo")
            nc.sync.dma_start(wk[:], wkT_d[:])
            nc.sync.dma_start(wv[:], wvT_d[:])
            for k2 in range(NM):
                nc.sync.dma_start(wo[:, k2, :], woT_d[:, k2, :])
            rnT = apool.tile([P, NKC, T], BF, tag="rnT")
            qT = apool.tile([P, NM, T], BF, tag="qT")
            # Scores stationary: heads a/b on partition halves; scores
            # matmuls are 2x row-tiled (K=64, tiles (0,0) and (64,0)) and run
            # concurrently on the PE.
            kT = apool.tile([P, NM, TC], BF, tag="kT")
            vv = apool.tile([P, NJ, HL * (HD + 1)], BF, tag="vv")
            attnU = apool.tile([P, NM, T], BF, tag="attnU")
            attnT = apool.tile([P, NM, T], BF, tag="attnT")
            # denominator rows live at 32-aligned partitions (BIR requires
            # engine APs to start on partition multiples of 32):
            # row(i, hh) -> partition 32*(2*(i%2)+hh), column half i//2
            denp = apool.tile([P, 2 * T], FP, tag="denp")
            recipp = apool.tile([P, 2 * T], FP, tag="recipp")
            recipb = apool.tile([P, 2 * T], BF, tag="recipb")

            # one-time zero/one fills (DVE, overlapped with initial DMAs)
            nc.gpsimd.memset(denp[:], 1.0)
            # ones column of v' (denominator rows)
            nc.vector.memset(
                vv[:].rearrange("p j (h c) -> p (j h) c", c=HD + 1)[:, :, HD : HD + 1],
                1.0,
            )
            # preload the Exp activation table before the attention phase
            dummy = cpool.tile([P, 1], BF)

            with (
                tc.tile_pool(name="psmm", bufs=2, space="PSUM") as psmm,
                tc.tile_pool(name="ctx", bufs=1) as ctxpool,
            ):
                # ---- phase A1: rownorm + transpose ----
                with (
                    tc.tile_pool(name="xrn", bufs=10) as xpool,
                    tc.tile_pool(name="xst", bufs=6) as spool,
                    tc.tile_pool(name="sqp", bufs=2) as sqpool,
                    tc.tile_pool(name="wqp", bufs=1) as wqpool,
                    tc.tile_pool(name="pstr", bufs=2, space="PSUM") as pstr,
                ):
                    xts = {}
                    for t in range(NT):
                        xts[t] = xpool.tile([P, C], BF, tag="xt", name=f"xt{t}")
                        nc.sync.dma_start(xts[t][:, 0:512], x_d[t][:, 0:512])
                        nc.sync.dma_start(xts[t][:, 512:1024], x_d[t][:, 512:1024])
                    wq = wqpool.tile([P, NKC, CL], BF, tag="wq")
                    nc.sync.dma_start(wq[:], wqT_d[:])
                    ctx8 = ctxpool.tile([P, NKC, TC], F8, tag="ctxT")
                    for k in range(NKC):
                        for hf in range(2):
                            nc.sync.dma_start(
                                ctx8[:, k, hf * 1024 : (hf + 1) * 1024],
                                ctxT_d[:, k, hf * 1024 : (hf + 1) * 1024],
                            )
                    for t in range(NT):
                        xt = xts[t]
                        nmu = spool.tile([P, 1], FP, tag="nmu")
                        nc.vector.reduce_sum(nmu[:], xt[:], axis=mybir.AxisListType.X)
                        nc.scalar.mul(nmu[:], nmu[:], -1.0 / C)
                        sq = sqpool.tile([P, C], BF, tag="sq")
                        ex2 = spool.tile([P, 1], FP, tag="ex2")
                        nc.scalar.activation(
                            sq[:], xt[:], mybir.ActivationFunctionType.Square,
                            accum_out=ex2[:],
                        )
                        var = spool.tile([P, 1], FP, tag="var")
                        nc.scalar.mul(ex2[:], ex2[:], 1.0 / C)
                        mu2 = spool.tile([P, 1], FP, tag="mu2")
                        nc.vector.tensor_mul(mu2[:], nmu[:], nmu[:])
                        nc.vector.tensor_sub(var[:], ex2[:], mu2[:])
                        std = spool.tile([P, 1], FP, tag="std")
                        nc.scalar.activation(
                            std[:], var[:], mybir.ActivationFunctionType.Sqrt,
                            bias=eps[:],
                        )
                        inv = spool.tile([P, 1], FP, tag="inv")
                        nc.vector.reciprocal(inv[:], std[:])
                        rn = xpool.tile([P, C], BF, tag="rn")
                        nc.vector.scalar_tensor_tensor(
                            out=rn[:], in0=xt[:], scalar=nmu[:],
                            in1=inv[:].to_broadcast((P, C)),
                            op0=mybir.AluOpType.add, op1=mybir.AluOpType.mult,
                        )
                        # 4 transposes per PSUM tile, one evacuation copy each
                        for c4 in range(2):
                            pt = pstr.tile([P, 512], BF, tag="ptr")
                            for cc in range(4):
                                c = 4 * c4 + cc
                                nc.tensor.transpose(
                                    pt[:, cc * P : (cc + 1) * P],
                                    rn[:, c * P : (c + 1) * P],
                                    ident[:],
                                )
                            nc.vector.tensor_copy(
                                rnT[:, 4 * c4 : 4 * c4 + 4, t * P : (t + 1) * P],
                                pt[:].rearrange("p (c q) -> p c q", q=P),
                            )

                    # ---- q projection: qT[m] += wq[k,m]^T @ rnT[k] ----
                    for m in range(NM):
                        ps = psmm.tile([P, T], FP, tag="mm")
                        for n in range(2):
                            ns = slice(n * 512, (n + 1) * 512)
                            for k in range(NKC):
                                nc.tensor.matmul(
                                    ps[:, ns],
                                    wq[:, k, m * P : (m + 1) * P],
                                    rnT[:, k, ns],
                                    start=(k == 0), stop=(k == NKC - 1),
                                )
                        nc.vector.tensor_scalar(
                            out=qT[:, m, :], in0=ps[:],
                            scalar1=cb[:, m : m + 1], scalar2=cb[:, 8:9],
                            op0=mybir.AluOpType.add, op1=mybir.AluOpType.mult,
                        )
                    # preload exp table after the last Sqrt (input depends
                    # on qT so the scheduler cannot hoist it before the LN)
                    nc.scalar.activation(
                        dummy[:], qT[:, 0, 0:1], mybir.ActivationFunctionType.Exp,
                    )

                # ---- k/v projections, context streamed in halves ----
                if True:
                    DR = mybir.MatmulPerfMode.DoubleRow
                    for ch in range(2):
                        hs = slice(ch * (TC // 2), (ch + 1) * (TC // 2))
                        # k projection for this context half (fp8 DoubleRow)
                        for m in range(NM):
                            ps = psmm.tile([P, TC // 2], FP, tag="mm")
                            for n in range(2):
                                ns = slice(ch * 1024 + n * 512, ch * 1024 + (n + 1) * 512)
                                for k in range(NKC // 2):
                                    nc.tensor.matmul(
                                        ps[:, n * 512 : (n + 1) * 512],
                                        wk[:, 2 * k : 2 * k + 2, m * P : (m + 1) * P],
                                        ctx8[:, 2 * k : 2 * k + 2, ns],
                                        start=(k == 0), stop=(k == NKC // 2 - 1),
                                        perf_mode=DR,
                                    )
                            nc.vector.tensor_scalar_add(
                                kT[:, m, hs], ps[:], cb[:, 4 + m : 5 + m],
                            )
                        # v projection for this context half (fp8 DoubleRow)
                        for jj in range(NJ // 2):
                            j = ch * (NJ // 2) + jj
                            ps = psmm.tile([P, CL], FP, tag="mmv")
                            for k in range(NKC // 2):
                                nc.tensor.matmul(
                                    ps[:],
                                    ctx8[:, 2 * k : 2 * k + 2, j * P : (j + 1) * P],
                                    wv[:, 2 * k : 2 * k + 2, :],
                                    start=(k == 0), stop=(k == NKC // 2 - 1),
                                    perf_mode=DR,
                                )
                            nc.vector.tensor_copy(
                                vv[:, j, :].rearrange("p (h c) -> p h c", c=HD + 1)[:, :, 0:HD],
                                ps[:].rearrange("p (h c) -> p h c", c=HD),
                            )

            # ---- attention: scores -> exp (ACT/DVE split) -> attn-out ----
            with (
                tc.tile_pool(name="expa", bufs=3) as eapool,
                tc.tile_pool(name="expb", bufs=3) as ebpool,
            ):
                with (
                    tc.tile_pool(name="pssc", bufs=2, space="PSUM") as pssc,
                    tc.tile_pool(name="psat", bufs=2, space="PSUM") as psat,
                ):
                    for i in range(NM):
                        ph = {}
                        for hh in range(2):
                            ph[hh] = psat.tile(
                                [HD + 1, T], FP, tag="ph", name=f"ph_{i}_{hh}",
                            )
                        ao_pending = []
                        for j in range(NJ):
                            js = slice(j * P, (j + 1) * P)
                            psc_a = pssc.tile([P, T], FP, tag="sc", name=f"sa{i}_{j}")
                            psc_b = pssc.tile([P, T], FP, tag="sc", name=f"sb{i}_{j}")
                            for n in range(2):
                                ns = slice(n * 512, (n + 1) * 512)
                                nc.tensor.matmul(psc_a[:, ns], kT[0:64, i, js],
                                                 qT[0:64, i, ns],
                                                 start=True, stop=True)
                                nc.tensor.matmul(psc_b[:, ns], kT[64:128, i, js],
                                                 qT[64:128, i, ns],
                                                 start=True, stop=True)
                            et_a = eapool.tile([P, T], BF, tag="eta", name=f"ea{i}_{j}")
                            nc.scalar.activation(
                                et_a[:], psc_a[:], mybir.ActivationFunctionType.Exp,
                            )
                            if j % 8 != 7:
                                et_b = ebpool.tile([P, T], I16, tag="etb", name=f"eb{i}_{j}")
                                nc.vector.tensor_scalar(
                                    out=et_b[:], in0=psc_b[:],
                                    scalar1=EXP_A, scalar2=EXP_B,
                                    op0=mybir.AluOpType.mult, op1=mybir.AluOpType.add,
                                )
                                et_b_bf = et_b[:].bitcast(BF)
                            else:
                                et_b2 = eapool.tile([P, T], BF, tag="eta", name=f"eb{i}_{j}")
                                nc.scalar.activation(
                                    et_b2[:], psc_b[:], mybir.ActivationFunctionType.Exp,
                                )
                                et_b_bf = et_b2[:]
                            ao_pending.append((j, et_a, et_b_bf))
                            if j >= 1:
                                jp, pa, pb = ao_pending.pop(0)
                                vs = vv[:, jp, :].rearrange("p (h c) -> p h c", c=HD + 1)
                                for n in range(2):
                                    ns = slice(n * 512, (n + 1) * 512)
                                    nc.tensor.matmul(
                                        ph[0][:, ns], vs[:, 2 * i, :], pa[:, ns],
                                        start=(jp == 0), stop=(jp == NJ - 1),
                                    )
                                    nc.tensor.matmul(
                                        ph[1][:, ns], vs[:, 2 * i + 1, :],
                                        pb[:, ns],
                                        start=(jp == 0), stop=(jp == NJ - 1),
                                    )
                        jp, pa, pb = ao_pending.pop(0)
                        vs = vv[:, jp, :].rearrange("p (h c) -> p h c", c=HD + 1)
                        for n in range(2):
                            ns = slice(n * 512, (n + 1) * 512)
                            nc.tensor.matmul(ph[0][:, ns], vs[:, 2 * i, :], pa[:, ns],
                                             start=(jp == 0), stop=(jp == NJ - 1))
                            nc.tensor.matmul(ph[1][:, ns], vs[:, 2 * i + 1, :],
                                             pb[:, ns],
                                             start=(jp == 0), stop=(jp == NJ - 1))
                        # evacuate: data rows -> attnU (ACT), denom rows -> denp (DVE)
                        for hh in range(2):
                            nc.scalar.copy(
                                attnU[64 * hh : 64 * hh + 64, i, :], ph[hh][0:64, :],
                            )
                            dp = 32 * (2 * (i % 2) + hh)
                            dc = (i // 2) * T
                            nc.vector.tensor_copy(
                                denp[dp : dp + 1, dc : dc + T], ph[hh][64:65, :],
                            )
                        if i == 1:
                            nc.vector.reciprocal_approx_fast(
                                recipp[:, 0:T], denp[:, 0:T],
                            )
                            nc.vector.tensor_copy(recipb[:, 0:T], recipp[:, 0:T])

                # ---- deferred normalize ----
                with tc.tile_pool(name="psel", bufs=2, space="PSUM") as psel:
                    nc.vector.reciprocal_approx_fast(
                        recipp[:, T : 2 * T], denp[:, T : 2 * T],
                    )
                    nc.vector.tensor_copy(recipb[:, T : 2 * T], recipp[:, T : 2 * T])
                    for i in range(NM):
                        dc = (i // 2) * T
                        rb = psel.tile([P, T], FP, tag="rb")
                        for n in range(2):
                            ns = slice(n * 512, (n + 1) * 512)
                            nc.tensor.matmul(
                                rb[:, ns], sel[:, i, :],
                                recipb[:, dc + n * 512 : dc + (n + 1) * 512],
                                start=True, stop=True,
                            )
                        nc.vector.tensor_mul(
                            attnT[:, i, :], attnU[:, i, :], rb[:],
                        )

                # ---- out-proj partials ----
                with tc.tile_pool(name="psoc", bufs=3, space="PSUM") as psoc:
                    with tc.tile_pool(name="oev", bufs=3) as opool:
                        for m in range(C // P):
                            po = psoc.tile([P, T], FP, tag="oc")
                            for n in range(2):
                                ns = slice(n * 512, (n + 1) * 512)
                                for k2 in range(NM):
                                    nc.tensor.matmul(
                                        po[:, ns],
                                        wo[:, k2, m * P : (m + 1) * P],
                                        attnT[:, k2, ns],
                                        start=(k2 == 0), stop=(k2 == NM - 1),
                                    )
                            ot = opool.tile([P, T], BF, tag="ot")
                            for n in range(2):
                                ns = slice(n * 512, (n + 1) * 512)
                                nc.vector.tensor_copy(ot[:, ns], po[:, ns])
                                nc.sync.dma_start(part_d[m][:, ns], ot[:, ns])

    nc.finalize()
    return nc


_NC_CACHE = {}


def _get_nc():
    if "nc" not in _NC_CACHE:
        _NC_CACHE["nc"] = _build_nc()
    return _NC_CACHE["nc"]


def _quant(w):
    g = np.float32(np.mean(np.abs(w), dtype=np.float64))
    t = np.clip(np.rint(w / (g + np.float32(Q_EPS))), -1.0, 1.0).astype(np.float32)
    return t, g


def _pack_kp(a):
    # [K, M] -> [P, K//P, M] (partition-major chunks)
    k, m = a.shape
    return np.ascontiguousarray(a.reshape(k // P, P, m).transpose(1, 0, 2))


def _bf(a):
    return np.ascontiguousarray(a.astype(ml_dtypes.bfloat16))


def _f8(a):
    return np.ascontiguousarray(a.astype(ml_dtypes.float8_e4m3))


def kernel(**inputs):
    global last_exec_time_ns
    x = np.asarray(inputs["x"], dtype=np.float32)
    ctx = np.asarray(inputs["context"], dtype=np.float32)
    Wq = np.asarray(inputs["Wq"], dtype=np.float32)
    Wk = np.asarray(inputs["Wk"], dtype=np.float32)
    Wv = np.asarray(inputs["Wv"], dtype=np.float32)
    Wo = np.asarray(inputs["Wo"], dtype=np.float32)
    bq = np.asarray(inputs["bq"], dtype=np.float32)
    bk = np.asarray(inputs["bk"], dtype=np.float32)
    bv = np.asarray(inputs["bv"], dtype=np.float32)
    bo = np.asarray(inputs["bo"], dtype=np.float32)
    g_ln = np.asarray(inputs["ln_gamma"], dtype=np.float32)
    b_ln = np.asarray(inputs["ln_beta"], dtype=np.float32)

    Tq, gq = _quant(Wq)
    Tk, gk = _quant(Wk)
    Tv, gv = _quant(Wv)
    To, go = _quant(Wo)

    qb_full = (bq + b_ln @ (gq * Tq).T) / gq          # [C]
    scale = np.float32(gq * gk * SCALE)
    host_bias = bo + bv @ (go * To).T                 # [C]

    # select matrices for the denominator broadcast: recipp partition
    # 32*(2*(i%2)+hh) feeds partitions [64*hh, 64*hh+64) of attnT chunk i
    selm = np.zeros((P, NM, P), dtype=np.float32)
    for i in range(NM):
        selm[32 * (2 * (i % 2)), i, 0:64] = 1.0
        selm[32 * (2 * (i % 2) + 1), i, 64:128] = 1.0

    in_maps = []
    for core in range(NCORES):
        b = core // 2
        g = core % 2
        rows = slice(CL * g, CL * (g + 1))
        wqT = _pack_kp((Tq[rows] * g_ln[None, :]).T)  # [P, 8, 512]
        wkT = _pack_kp(Tk[rows].T)
        wvT = _pack_kp(Tv[rows].T)
        woT = _pack_kp((To[:, rows] * (go * gv)).T)   # [P, 4, 1024]
        cbm = np.zeros((P, 9), dtype=np.float32)
        cbm[:, 0:4] = qb_full[rows].reshape(4, P).T
        cbm[:, 4:8] = (bk[rows] / gk).reshape(4, P).T
        cbm[:, 8] = scale
        in_maps.append({
            "x": _bf(x[b].reshape(T // P, P, C)),
            "ctxT": _f8(_pack_kp(np.ascontiguousarray(ctx[b].T))),
            "wqT": _bf(wqT), "wkT": _f8(wkT), "wvT": _f8(wvT), "woT": _bf(woT),
            "cb": cbm,
            "sel": _bf(selm),
        })

    nc = _get_nc()
    trace = os.environ.get("KERNEL_TRACE", "0") == "1"
    res = run_bass_kernel_spmd(nc, in_maps, list(range(NCORES)), trace=trace)
    last_exec_time_ns = res.exec_time_ns

    out = np.empty((B, T, C), dtype=np.float32)
    for b in range(B):
        p0 = res.results[2 * b]["partial"].astype(np.float32).reshape(C, T)
        p1 = res.results[2 * b + 1]["partial"].astype(np.float32).reshape(C, T)
        out[b] = x[b] + p0.T + p1.T + host_bias[None, :]
    return out


# revision 18
# speedup vs baseline: 1.1319x; 1.1319x over previous
"""Cross-modal attention block on 8 Trainium2 NeuronCores.

Sharding: core = 2*b + g  ->  batch b (4-way data parallel) x head-group g
(2-way tensor parallel over 16 heads -> 8 heads/core).  Each core:
  rownorm(x[b]) -> PE transpose -> q projection (ternary weights, gamma/beta
  folded) ; kT/v projections from pre-transposed context ; per-head
  scoresT = k~^T q~ ; exp split between ScalarE (exact) and VectorE
  (Schraudolph bit-trick into bf16) ; unnormalized attn-out with an appended
  ones-row producing softmax denominators in the same matmul ; deferred
  batch normalize (reciprocal_approx_fast + select-matmul broadcast) ;
  out-proj partial.  Host sums the two partials per batch + residual +
  folded biases.

All matmuls are full 128x128-mode bf16 (scores use zero-padded K so the PE
never enters a tiled mode, which measures as HAM-throttled 1.2 GHz).
"""

import os

import ml_dtypes
import numpy as np

import concourse.bass as bass
import concourse.mybir as mybir
import concourse.tile as tile
from concourse import bacc
from concourse.bass_utils import run_bass_kernel_spmd
from concourse.masks import make_identity

FP = mybir.dt.float32
FPR = mybir.dt.float32r
BF = mybir.dt.bfloat16
I16 = mybir.dt.int16
F8 = mybir.dt.float8e4

B, T, TC, C = 4, 1024, 2048, 1024
H, HD = 16, 64
HL = 8           # heads per core
CL = HL * HD     # 512 local channels
SCALE = HD ** -0.5
LN_EPS = 1e-5
Q_EPS = 1e-5
P = 128
NCORES = 8

NT = T // P      # 8 query-row tiles
NKC = C // P     # 8 contraction chunks over C
NJ = TC // P     # 16 context chunks
NM = CL // P     # 4 local d-chunks

# Schraudolph fast-exp into bf16 bit pattern via int16:
#   i16 = trunc(x * EXP_A + EXP_B); bf16 = bits(i16)
# max rel err ~3.3% over x in [-10, 8]; scores*scale stay well inside.
EXP_A = float(np.float32(128.0 / np.log(2.0)))
EXP_B = float(np.float32(16256.0 - 5.1))

last_exec_time_ns = None


def _build_nc():
    nc = bacc.Bacc(None, target_bir_lowering=False, debug=False)

    x_d = nc.dram_tensor("x", [NT, P, C], BF, kind="ExternalInput")
    ctxT_d = nc.dram_tensor("ctxT", [P, NKC, TC], F8, kind="ExternalInput")
    wqT_d = nc.dram_tensor("wqT", [P, NKC, CL], F8, kind="ExternalInput")
    wkT_d = nc.dram_tensor("wkT", [P, NKC, CL], F8, kind="ExternalInput")
    wvT_d = nc.dram_tensor("wvT", [P, NKC, CL], F8, kind="ExternalInput")
    woT_d = nc.dram_tensor("woT", [P, NM, C], F8, kind="ExternalInput")
    cb_d = nc.dram_tensor("cb", [P, 9], FP, kind="ExternalInput")
    sel_d = nc.dram_tensor("sel", [P, NM, P], BF, kind="ExternalInput")
    part_d = nc.dram_tensor("partial", [C // P, P, T], BF, kind="ExternalOutput")

    with tile.TileContext(nc) as tc:
        with (
            tc.tile_pool(name="const", bufs=1) as cpool,
            tc.tile_pool(name="acts", bufs=1) as apool,
        ):
            ident_f = cpool.tile([P, P], FP)
            make_identity(nc, ident_f[:])
            ident = cpool.tile([P, P], BF)
            nc.vector.tensor_copy(ident[:], ident_f[:])
            cb = cpool.tile([P, 9], FP)
            nc.sync.dma_start(cb[:], cb_d[:])
            sel = cpool.tile([P, NM, P], BF)
            nc.sync.dma_start(sel[:], sel_d[:])
            eps = cpool.tile([P, 1], FP)
            nc.vector.memset(eps[:], LN_EPS)

            wk = apool.tile([P, NKC, CL], F8, tag="wk")
            wv = apool.tile([P, NKC, CL], F8, tag="wv")
            wo = apool.tile([P, NM, C], F8, tag="wo")
            nc.sync.dma_start(wk[:], wkT_d[:])
            nc.sync.dma_start(wv[:], wvT_d[:])
            for k2 in range(NM):
                nc.sync.dma_start(wo[:, k2, :], woT_d[:, k2, :])
            rnT = apool.tile([P, NKC, T], F8, tag="rnT")
            qT = apool.tile([P, NM, T], BF, tag="qT")
            # Scores stationaries, zero-padded so every matmul is full K=128:
            # kTa rows 0-63 = head-a k rows (rows 64-127 zero), kTb vice versa.
            kTa = apool.tile([P, NM, TC], BF, tag="kTa")
            kTb = apool.tile([P, NM, TC], BF, tag="kTb")
            vv = apool.tile([P, NJ, HL * (HD + 1)], BF, tag="vv")
            attnU = apool.tile([P, NM, T], BF, tag="attnU")
            attnT = apool.tile([P, NM, T], F8, tag="attnT")
            # denominator rows live at 32-aligned partitions (BIR requires
            # engine APs to start on partition multiples of 32):
            # row(i, hh) -> partition 32*(2*(i%2)+hh), column half i//2
            denp = apool.tile([P, 2 * T], FP, tag="denp")
            recipp = apool.tile([P, 2 * T], FP, tag="recipp")
            recipb = apool.tile([P, 2 * T], BF, tag="recipb")

            # one-time zero/one fills (DVE, overlapped with initial DMAs)
            nc.gpsimd.memset(kTa[64:128, :, :], 0.0)
            nc.gpsimd.memset(kTb[0:64, :, :], 0.0)
            nc.gpsimd.memset(denp[:], 1.0)
            # ones column of v' (denominator rows)
            nc.vector.memset(
                vv[:].rearrange("p j (h c) -> p (j h) c", c=HD + 1)[:, :, HD : HD + 1],
                1.0,
            )
            # preload the Exp activation table before the attention phase
            dummy = cpool.tile([P, 1], BF)

            with (
                tc.tile_pool(name="psmm", bufs=2, space="PSUM") as psmm,
                tc.tile_pool(name="ctx", bufs=1) as ctxpool,
            ):
                # ---- phase A1: rownorm + transpose ----
                with (
                    tc.tile_pool(name="xrn", bufs=10) as xpool,
                    tc.tile_pool(name="xst", bufs=6) as spool,
                    tc.tile_pool(name="sqp", bufs=2) as sqpool,
                    tc.tile_pool(name="wqp", bufs=1) as wqpool,
                    tc.tile_pool(name="pstr", bufs=2, space="PSUM") as pstr,
                ):
                    xts = {}
                    for t in range(NT):
                        xts[t] = xpool.tile([P, C], BF, tag="xt", name=f"xt{t}")
                        nc.sync.dma_start(xts[t][:, 0:512], x_d[t][:, 0:512])
                        nc.sync.dma_start(xts[t][:, 512:1024], x_d[t][:, 512:1024])
                    wq = wqpool.tile([P, NKC, CL], F8, tag="wq")
                    nc.sync.dma_start(wq[:], wqT_d[:])
                    ctx8 = ctxpool.tile([P, NKC, TC], F8, tag="ctxT")
                    for k in range(NKC):
                        for hf in range(2):
                            nc.sync.dma_start(
                                ctx8[:, k, hf * 1024 : (hf + 1) * 1024],
                                ctxT_d[:, k, hf * 1024 : (hf + 1) * 1024],
                            )
                    for t in range(NT):
                        xt = xts[t]
                        nmu = spool.tile([P, 1], FP, tag="nmu")
                        nc.vector.reduce_sum(nmu[:], xt[:], axis=mybir.AxisListType.X)
                        nc.scalar.mul(nmu[:], nmu[:], -1.0 / C)
                        sq = sqpool.tile([P, C], BF, tag="sq")
                        ex2 = spool.tile([P, 1], FP, tag="ex2")
                        nc.scalar.activation(
                            sq[:], xt[:], mybir.ActivationFunctionType.Square,
                            accum_out=ex2[:],
                        )
                        var = spool.tile([P, 1], FP, tag="var")
                        nc.scalar.mul(ex2[:], ex2[:], 1.0 / C)
                        mu2 = spool.tile([P, 1], FP, tag="mu2")
                        nc.vector.tensor_mul(mu2[:], nmu[:], nmu[:])
                        nc.vector.tensor_sub(var[:], ex2[:], mu2[:])
                        std = spool.tile([P, 1], FP, tag="std")
                        nc.scalar.activation(
                            std[:], var[:], mybir.ActivationFunctionType.Sqrt,
                            bias=eps[:],
                        )
                        inv = spool.tile([P, 1], FP, tag="inv")
                        nc.vector.reciprocal(inv[:], std[:])
                        rn = xpool.tile([P, C], BF, tag="rn")
                        nc.vector.scalar_tensor_tensor(
                            out=rn[:], in0=xt[:], scalar=nmu[:],
                            in1=inv[:].to_broadcast((P, C)),
                            op0=mybir.AluOpType.add, op1=mybir.AluOpType.mult,
                        )
                        # 4 transposes per PSUM tile, one evacuation copy each
                        for c4 in range(2):
                            pt = pstr.tile([P, 512], BF, tag="ptr")
                            for cc in range(4):
                                c = 4 * c4 + cc
                                nc.tensor.transpose(
                                    pt[:, cc * P : (cc + 1) * P],
                                    rn[:, c * P : (c + 1) * P],
                                    ident[:],
                                )
                            nc.vector.tensor_copy(
                                rnT[:, 4 * c4 : 4 * c4 + 4, t * P : (t + 1) * P],
                                pt[:].rearrange("p (c q) -> p c q", q=P),
                            )

                    # ---- q projection: qT[m] += wq[k,m]^T @ rnT[k] ----
                    DRQ = mybir.MatmulPerfMode.DoubleRow
                    for m in range(NM):
                        ps = psmm.tile([P, T], FP, tag="mm")
                        for n in range(2):
                            ns = slice(n * 512, (n + 1) * 512)
                            for k in range(NKC // 2):
                                nc.tensor.matmul(
                                    ps[:, ns],
                                    wq[:, 2 * k : 2 * k + 2, m * P : (m + 1) * P],
                                    rnT[:, 2 * k : 2 * k + 2, ns],
                                    start=(k == 0), stop=(k == NKC // 2 - 1),
                                    perf_mode=DRQ,
                                )
                        nc.vector.tensor_scalar(
                            out=qT[:, m, :], in0=ps[:],
                            scalar1=cb[:, m : m + 1], scalar2=cb[:, 8:9],
                            op0=mybir.AluOpType.add, op1=mybir.AluOpType.mult,
                        )
                    # preload exp table after the last Sqrt (input depends
                    # on qT so the scheduler cannot hoist it before the LN)
                    nc.scalar.activation(
                        dummy[:], qT[:, 0, 0:1], mybir.ActivationFunctionType.Exp,
                    )

                # ---- k/v projections, context streamed in halves ----
                if True:
                    DR = mybir.MatmulPerfMode.DoubleRow
                    for ch in range(2):
                        hs = slice(ch * (TC // 2), (ch + 1) * (TC // 2))
                        # k projection for this context half (fp8 DoubleRow)
                        for m in range(NM):
                            ps = psmm.tile([P, TC // 2], FP, tag="mm")
                            for n in range(2):
                                ns = slice(ch * 1024 + n * 512, ch * 1024 + (n + 1) * 512)
                                for k in range(NKC // 2):
                                    nc.tensor.matmul(
                                        ps[:, n * 512 : (n + 1) * 512],
                                        wk[:, 2 * k : 2 * k + 2, m * P : (m + 1) * P],
                                        ctx8[:, 2 * k : 2 * k + 2, ns],
                                        start=(k == 0), stop=(k == NKC // 2 - 1),
                                        perf_mode=DR,
                                    )
                            nc.vector.tensor_scalar_add(
                                kTa[0:64, m, hs], ps[0:64, :], cb[0:64, 4 + m : 5 + m],
                            )
                            nc.vector.tensor_scalar_add(
                                kTb[64:128, m, hs], ps[64:128, :], cb[64:128, 4 + m : 5 + m],
                            )
                        # v projection for this context half (fp8 DoubleRow)
                        for jj in range(NJ // 2):
                            j = ch * (NJ // 2) + jj
                            ps = psmm.tile([P, CL], FP, tag="mmv")
                            for k in range(NKC // 2):
                                nc.tensor.matmul(
                                    ps[:],
                                    ctx8[:, 2 * k : 2 * k + 2, j * P : (j + 1) * P],
                                    wv[:, 2 * k : 2 * k + 2, :],
                                    start=(k == 0), stop=(k == NKC // 2 - 1),
                                    perf_mode=DR,
                                )
                            nc.vector.tensor_copy(
                                vv[:, j, :].rearrange("p (h c) -> p h c", c=HD + 1)[:, :, 0:HD],
                                ps[:].rearrange("p (h c) -> p h c", c=HD),
                            )

            # ---- attention: scores -> exp (ACT/DVE split) -> attn-out ----
            with (
                tc.tile_pool(name="expa", bufs=3) as eapool,
                tc.tile_pool(name="expb", bufs=3) as ebpool,
            ):
                with (
                    tc.tile_pool(name="pssc", bufs=2, space="PSUM") as pssc,
                    tc.tile_pool(name="psat", bufs=2, space="PSUM") as psat,
                ):
                    for i in range(NM):
                        ph = {}
                        for hh in range(2):
                            ph[hh] = psat.tile(
                                [HD + 1, T], FP, tag="ph", name=f"ph_{i}_{hh}",
                            )
                        ao_pending = []
                        for j in range(NJ):
                            js = slice(j * P, (j + 1) * P)
                            psc_a = pssc.tile([P, T], FP, tag="sc", name=f"sa{i}_{j}")
                            psc_b = pssc.tile([P, T], FP, tag="sc", name=f"sb{i}_{j}")
                            for n in range(2):
                                ns = slice(n * 512, (n + 1) * 512)
                                nc.tensor.matmul(psc_a[:, ns], kTa[:, i, js],
                                                 qT[:, i, ns],
                                                 start=True, stop=True)
                                nc.tensor.matmul(psc_b[:, ns], kTb[:, i, js],
                                                 qT[:, i, ns],
                                                 start=True, stop=True)
                            et_a = eapool.tile([P, T], BF, tag="eta", name=f"ea{i}_{j}")
                            nc.scalar.activation(
                                et_a[:], psc_a[:], mybir.ActivationFunctionType.Exp,
                            )
                            if j % 4 != 3:
                                et_b = ebpool.tile([P, T], I16, tag="etb", name=f"eb{i}_{j}")
                                nc.vector.tensor_scalar(
                                    out=et_b[:], in0=psc_b[:],
                                    scalar1=EXP_A, scalar2=EXP_B,
                                    op0=mybir.AluOpType.mult, op1=mybir.AluOpType.add,
                                )
                                et_b_bf = et_b[:].bitcast(BF)
                            else:
                                et_b2 = eapool.tile([P, T], BF, tag="eta", name=f"eb{i}_{j}")
                                nc.scalar.activation(
                                    et_b2[:], psc_b[:], mybir.ActivationFunctionType.Exp,
                                )
                                et_b_bf = et_b2[:]
                            ao_pending.append((j, et_a, et_b_bf))
                            if j >= 1:
                                jp, pa, pb = ao_pending.pop(0)
                                vs = vv[:, jp, :].rearrange("p (h c) -> p h c", c=HD + 1)
                                for n in range(2):
                                    ns = slice(n * 512, (n + 1) * 512)
                                    nc.tensor.matmul(
                                        ph[0][:, ns], vs[:, 2 * i, :], pa[:, ns],
                                        start=(jp == 0), stop=(jp == NJ - 1),
                                    )
                                    nc.tensor.matmul(
                                        ph[1][:, ns], vs[:, 2 * i + 1, :],
                                        pb[:, ns],
                                        start=(jp == 0), stop=(jp == NJ - 1),
                                    )
                        jp, pa, pb = ao_pending.pop(0)
                        vs = vv[:, jp, :].rearrange("p (h c) -> p h c", c=HD + 1)
                        for n in range(2):
                            ns = slice(n * 512, (n + 1) * 512)
                            nc.tensor.matmul(ph[0][:, ns], vs[:, 2 * i, :], pa[:, ns],
                                             start=(jp == 0), stop=(jp == NJ - 1))
                            nc.tensor.matmul(ph[1][:, ns], vs[:, 2 * i + 1, :],
                                             pb[:, ns],
                                             start=(jp == 0), stop=(jp == NJ - 1))
                        # evacuate: data rows -> attnU (ACT), denom rows -> denp (DVE)
                        for hh in range(2):
                            nc.scalar.copy(
                                attnU[64 * hh : 64 * hh + 64, i, :], ph[hh][0:64, :],
                            )
                            dp = 32 * (2 * (i % 2) + hh)
                            dc = (i // 2) * T
                            nc.vector.tensor_copy(
                                denp[dp : dp + 1, dc : dc + T], ph[hh][64:65, :],
                            )
                        if i == 1:
                            nc.vector.reciprocal_approx_fast(
                                recipp[:, 0:T], denp[:, 0:T],
                            )
                            nc.vector.tensor_copy(recipb[:, 0:T], recipp[:, 0:T])

                # ---- deferred normalize ----
                with tc.tile_pool(name="psel", bufs=2, space="PSUM") as psel:
                    nc.vector.reciprocal_approx_fast(
                        recipp[:, T : 2 * T], denp[:, T : 2 * T],
                    )
                    nc.vector.tensor_copy(recipb[:, T : 2 * T], recipp[:, T : 2 * T])
                    for i in range(NM):
                        dc = (i // 2) * T
                        rb = psel.tile([P, T], FP, tag="rb")
                        for n in range(2):
                            ns = slice(n * 512, (n + 1) * 512)
                            nc.tensor.matmul(
                                rb[:, ns], sel[:, i, :],
                                recipb[:, dc + n * 512 : dc + (n + 1) * 512],
                                start=True, stop=True,
                            )
                        nc.vector.tensor_mul(
                            attnT[:, i, :], attnU[:, i, :], rb[:],
                        )

                # ---- out-proj partials ----
                with tc.tile_pool(name="psoc", bufs=3, space="PSUM") as psoc:
                    with tc.tile_pool(name="oev", bufs=3) as opool:
                        for m in range(C // P):
                            po = psoc.tile([P, T], FP, tag="oc")
                            DRO = mybir.MatmulPerfMode.DoubleRow
                            for n in range(2):
                                ns = slice(n * 512, (n + 1) * 512)
                                for k2 in range(NM // 2):
                                    nc.tensor.matmul(
                                        po[:, ns],
                                        wo[:, 2 * k2 : 2 * k2 + 2, m * P : (m + 1) * P],
                                        attnT[:, 2 * k2 : 2 * k2 + 2, ns],
                                        start=(k2 == 0), stop=(k2 == NM // 2 - 1),
                                        perf_mode=DRO,
                                    )
                            ot = opool.tile([P, T], BF, tag="ot")
                            for n in range(2):
                                ns = slice(n * 512, (n + 1) * 512)
                                nc.vector.tensor_copy(ot[:, ns], po[:, ns])
                                nc.sync.dma_start(part_d[m][:, ns], ot[:, ns])

    nc.finalize()
    return nc


_NC_CACHE = {}


def _get_nc():
    if "nc" not in _NC_CACHE:
        _NC_CACHE["nc"] = _build_nc()
    return _NC_CACHE["nc"]


def _quant(w):
    g = np.float32(np.mean(np.abs(w), dtype=np.float64))
    t = np.clip(np.rint(w / (g + np.float32(Q_EPS))), -1.0, 1.0).astype(np.float32)
    return t, g


def _pack_kp(a):
    # [K, M] -> [P, K//P, M] (partition-major chunks)
    k, m = a.shape
    return np.ascontiguousarray(a.reshape(k // P, P, m).transpose(1, 0, 2))


def _bf(a):
    return np.ascontiguousarray(a.astype(ml_dtypes.bfloat16))


def _f8(a):
    return np.ascontiguousarray(a.astype(ml_dtypes.float8_e4m3))


def kernel(**inputs):
    global last_exec_time_ns
    x = np.asarray(inputs["x"], dtype=np.float32)
    ctx = np.asarray(inputs["context"], dtype=np.float32)
    Wq = np.asarray(inputs["Wq"], dtype=np.float32)
    Wk = np.asarray(inputs["Wk"], dtype=np.float32)
    Wv = np.asarray(inputs["Wv"], dtype=np.float32)
    Wo = np.asarray(inputs["Wo"], dtype=np.float32)
    bq = np.asarray(inputs["bq"], dtype=np.float32)
    bk = np.asarray(inputs["bk"], dtype=np.float32)
    bv = np.asarray(inputs["bv"], dtype=np.float32)
    bo = np.asarray(inputs["bo"], dtype=np.float32)
    g_ln = np.asarray(inputs["ln_gamma"], dtype=np.float32)
    b_ln = np.asarray(inputs["ln_beta"], dtype=np.float32)

    Tq, gq = _quant(Wq)
    Tk, gk = _quant(Wk)
    Tv, gv = _quant(Wv)
    To, go = _quant(Wo)

    qb_full = (bq + b_ln @ (gq * Tq).T) / gq          # [C]
    scale = np.float32(gq * gk * SCALE)
    host_bias = bo + bv @ (go * To).T                 # [C]

    # select matrices for the denominator broadcast: recipp partition
    # 32*(2*(i%2)+hh) feeds partitions [64*hh, 64*hh+64) of attnT chunk i
    selm = np.zeros((P, NM, P), dtype=np.float32)
    for i in range(NM):
        selm[32 * (2 * (i % 2)), i, 0:64] = 1.0
        selm[32 * (2 * (i % 2) + 1), i, 64:128] = 1.0

    in_maps = []
    for core in range(NCORES):
        b = core // 2
        g = core % 2
        rows = slice(CL * g, CL * (g + 1))
        wqT = _pack_kp((Tq[rows] * g_ln[None, :]).T)  # [P, 8, 512]
        wkT = _pack_kp(Tk[rows].T)
        wvT = _pack_kp(Tv[rows].T)
        woT = _pack_kp(To[:, rows].T)                 # [P, 4, 1024] ternary
        cbm = np.zeros((P, 9), dtype=np.float32)
        cbm[:, 0:4] = qb_full[rows].reshape(4, P).T
        cbm[:, 4:8] = (bk[rows] / gk).reshape(4, P).T
        cbm[:, 8] = scale
        in_maps.append({
            "x": _bf(x[b].reshape(T // P, P, C)),
            "ctxT": _f8(_pack_kp(np.ascontiguousarray(ctx[b].T))),
            "wqT": _f8(wqT), "wkT": _f8(wkT), "wvT": _f8(wvT), "woT": _f8(woT),
            "cb": cbm,
            "sel": _bf(selm),
        })

    nc = _get_nc()
    trace = os.environ.get("KERNEL_TRACE", "0") == "1"
    res = run_bass_kernel_spmd(nc, in_maps, list(range(NCORES)), trace=trace)
    last_exec_time_ns = res.exec_time_ns

    ogv = np.float32(go * gv)
    out = np.empty((B, T, C), dtype=np.float32)
    for b in range(B):
        p0 = res.results[2 * b]["partial"].astype(np.float32).reshape(C, T)
        p1 = res.results[2 * b + 1]["partial"].astype(np.float32).reshape(C, T)
        out[b] = x[b] + (p0.T + p1.T) * ogv + host_bias[None, :]
    return out


# revision 19
# speedup vs baseline: 1.1783x; 1.0410x over previous
"""Cross-modal attention block on 8 Trainium2 NeuronCores.

Sharding: core = 2*b + g  ->  batch b (4-way data parallel) x head-group g
(2-way tensor parallel over 16 heads -> 8 heads/core).  Each core:
  rownorm(x[b]) -> PE transpose -> q projection (ternary weights, gamma/beta
  folded) ; kT/v projections from pre-transposed context ; per-head
  scoresT = k~^T q~ ; exp split between ScalarE (exact) and VectorE
  (Schraudolph bit-trick into bf16) ; unnormalized attn-out with an appended
  ones-row producing softmax denominators in the same matmul ; deferred
  batch normalize (reciprocal_approx_fast + select-matmul broadcast) ;
  out-proj partial.  Host sums the two partials per batch + residual +
  folded biases.

All matmuls are full 128x128-mode bf16 (scores use zero-padded K so the PE
never enters a tiled mode, which measures as HAM-throttled 1.2 GHz).
"""

import os

import ml_dtypes
import numpy as np

import concourse.bass as bass
import concourse.mybir as mybir
import concourse.tile as tile
from concourse import bacc
from concourse.bass_utils import run_bass_kernel_spmd
from concourse.masks import make_identity

FP = mybir.dt.float32
FPR = mybir.dt.float32r
BF = mybir.dt.bfloat16
I16 = mybir.dt.int16
F8 = mybir.dt.float8e4

B, T, TC, C = 4, 1024, 2048, 1024
H, HD = 16, 64
HL = 8           # heads per core
CL = HL * HD     # 512 local channels
SCALE = HD ** -0.5
LN_EPS = 1e-5
Q_EPS = 1e-5
P = 128
NCORES = 8

NT = T // P      # 8 query-row tiles
NKC = C // P     # 8 contraction chunks over C
NJ = TC // P     # 16 context chunks
NM = CL // P     # 4 local d-chunks

# Schraudolph fast-exp into bf16 bit pattern via int16:
#   i16 = trunc(x * EXP_A + EXP_B); bf16 = bits(i16)
# max rel err ~3.3% over x in [-10, 8]; scores*scale stay well inside.
EXP_A = float(np.float32(128.0 / np.log(2.0)))
EXP_B = float(np.float32(16256.0 - 5.1))

last_exec_time_ns = None


def _build_nc():
    nc = bacc.Bacc(None, target_bir_lowering=False, debug=False)

    x_d = nc.dram_tensor("x", [NT, P, C], BF, kind="ExternalInput")
    ctxT_d = nc.dram_tensor("ctxT", [P, NKC, TC], F8, kind="ExternalInput")
    wqT_d = nc.dram_tensor("wqT", [P, NKC, CL], F8, kind="ExternalInput")
    wkT_d = nc.dram_tensor("wkT", [P, NKC, CL], F8, kind="ExternalInput")
    wvT_d = nc.dram_tensor("wvT", [P, NKC, CL], F8, kind="ExternalInput")
    woT_d = nc.dram_tensor("woT", [P, NM, C], F8, kind="ExternalInput")
    cb_d = nc.dram_tensor("cb", [P, 9], FP, kind="ExternalInput")
    sel_d = nc.dram_tensor("sel", [P, NM, P], BF, kind="ExternalInput")
    part_d = nc.dram_tensor("partial", [C // P, P, T], BF, kind="ExternalOutput")

    with tile.TileContext(nc) as tc:
        with (
            tc.tile_pool(name="const", bufs=1) as cpool,
            tc.tile_pool(name="acts", bufs=1) as apool,
        ):
            ident_f = cpool.tile([P, P], FP)
            make_identity(nc, ident_f[:])
            ident = cpool.tile([P, P], BF)
            nc.vector.tensor_copy(ident[:], ident_f[:])
            cb = cpool.tile([P, 9], FP)
            nc.sync.dma_start(cb[:], cb_d[:])
            sel = cpool.tile([P, NM, P], BF)
            nc.sync.dma_start(sel[:], sel_d[:])
            eps = cpool.tile([P, 1], FP)
            nc.vector.memset(eps[:], LN_EPS)

            wk = apool.tile([P, NKC, CL], F8, tag="wk")
            wv = apool.tile([P, NKC, CL], F8, tag="wv")
            wo = apool.tile([P, NM, C], F8, tag="wo")
            nc.sync.dma_start(wk[:], wkT_d[:])
            nc.sync.dma_start(wv[:], wvT_d[:])
            for k2 in range(NM):
                nc.sync.dma_start(wo[:, k2, :], woT_d[:, k2, :])
            rnT = apool.tile([P, NKC, T], F8, tag="rnT")
            qT = apool.tile([P, NM, T], BF, tag="qT")
            # Scores stationaries, zero-padded so every matmul is full K=128:
            # kTa rows 0-63 = head-a k rows (rows 64-127 zero), kTb vice versa.
            kTa = apool.tile([P, NM, TC], BF, tag="kTa")
            kTb = apool.tile([P, NM, TC], BF, tag="kTb")
            vv = apool.tile([P, NJ, HL * (HD + 1)], BF, tag="vv")
            attnU = apool.tile([P, NM, T], BF, tag="attnU")
            attnT = apool.tile([P, NM, T], F8, tag="attnT")
            # denominator rows live at 32-aligned partitions (BIR requires
            # engine APs to start on partition multiples of 32):
            # row(i, hh) -> partition 32*(2*(i%2)+hh), column half i//2
            denp = apool.tile([P, 2 * T], FP, tag="denp")
            recipp = apool.tile([P, 2 * T], FP, tag="recipp")
            recipb = apool.tile([P, 2 * T], BF, tag="recipb")

            # one-time zero/one fills (DVE, overlapped with initial DMAs)
            nc.gpsimd.memset(kTa[64:128, :, :], 0.0)
            nc.gpsimd.memset(kTb[0:64, :, :], 0.0)
            nc.gpsimd.memset(denp[:], 1.0)
            # ones column of v' (denominator rows)
            nc.vector.memset(
                vv[:].rearrange("p j (h c) -> p (j h) c", c=HD + 1)[:, :, HD : HD + 1],
                1.0,
            )
            # preload the Exp activation table before the attention phase
            dummy = cpool.tile([P, 1], BF)

            with (
                tc.tile_pool(name="psmm", bufs=2, space="PSUM") as psmm,
                tc.tile_pool(name="ctx", bufs=1) as ctxpool,
            ):
                # ---- phase A1: rownorm + transpose ----
                with (
                    tc.tile_pool(name="xrn", bufs=10) as xpool,
                    tc.tile_pool(name="xst", bufs=6) as spool,
                    tc.tile_pool(name="sqp", bufs=2) as sqpool,
                    tc.tile_pool(name="wqp", bufs=1) as wqpool,
                    tc.tile_pool(name="pstr", bufs=2, space="PSUM") as pstr,
                ):
                    xts = {}
                    for t in range(NT):
                        xts[t] = xpool.tile([P, C], BF, tag="xt", name=f"xt{t}")
                        nc.sync.dma_start(xts[t][:, 0:512], x_d[t][:, 0:512])
                        nc.sync.dma_start(xts[t][:, 512:1024], x_d[t][:, 512:1024])
                    wq = wqpool.tile([P, NKC, CL], F8, tag="wq")
                    nc.sync.dma_start(wq[:], wqT_d[:])
                    ctx8 = ctxpool.tile([P, NKC, TC], F8, tag="ctxT")
                    for k in range(NKC):
                        for hf in range(2):
                            nc.sync.dma_start(
                                ctx8[:, k, hf * 1024 : (hf + 1) * 1024],
                                ctxT_d[:, k, hf * 1024 : (hf + 1) * 1024],
                            )
                    for t in range(NT):
                        xt = xts[t]
                        nmu = spool.tile([P, 1], FP, tag="nmu")
                        nc.vector.reduce_sum(nmu[:], xt[:], axis=mybir.AxisListType.X)
                        nc.scalar.mul(nmu[:], nmu[:], -1.0 / C)
                        sq = sqpool.tile([P, C], BF, tag="sq")
                        ex2 = spool.tile([P, 1], FP, tag="ex2")
                        nc.scalar.activation(
                            sq[:], xt[:], mybir.ActivationFunctionType.Square,
                            accum_out=ex2[:],
                        )
                        var = spool.tile([P, 1], FP, tag="var")
                        nc.scalar.mul(ex2[:], ex2[:], 1.0 / C)
                        mu2 = spool.tile([P, 1], FP, tag="mu2")
                        nc.vector.tensor_mul(mu2[:], nmu[:], nmu[:])
                        nc.vector.tensor_sub(var[:], ex2[:], mu2[:])
                        std = spool.tile([P, 1], FP, tag="std")
                        nc.scalar.activation(
                            std[:], var[:], mybir.ActivationFunctionType.Sqrt,
                            bias=eps[:],
                        )
                        inv = spool.tile([P, 1], FP, tag="inv")
                        nc.vector.reciprocal(inv[:], std[:])
                        rn = xpool.tile([P, C], BF, tag="rn")
                        nc.vector.scalar_tensor_tensor(
                            out=rn[:], in0=xt[:], scalar=nmu[:],
                            in1=inv[:].to_broadcast((P, C)),
                            op0=mybir.AluOpType.add, op1=mybir.AluOpType.mult,
                        )
                        # 4 transposes per PSUM tile, one evacuation copy each
                        for c4 in range(2):
                            pt = pstr.tile([P, 512], BF, tag="ptr")
                            for cc in range(4):
                                c = 4 * c4 + cc
                                nc.tensor.transpose(
                                    pt[:, cc * P : (cc + 1) * P],
                                    rn[:, c * P : (c + 1) * P],
                                    ident[:],
                                )
                            nc.vector.tensor_copy(
                                rnT[:, 4 * c4 : 4 * c4 + 4, t * P : (t + 1) * P],
                                pt[:].rearrange("p (c q) -> p c q", q=P),
                            )

                    # ---- q projection: qT[m] += wq[k,m]^T @ rnT[k] ----
                    DRQ = mybir.MatmulPerfMode.DoubleRow
                    for m in range(NM):
                        ps = psmm.tile([P, T], FP, tag="mm")
                        for n in range(2):
                            ns = slice(n * 512, (n + 1) * 512)
                            for k in range(NKC // 2):
                                nc.tensor.matmul(
                                    ps[:, ns],
                                    wq[:, 2 * k : 2 * k + 2, m * P : (m + 1) * P],
                                    rnT[:, 2 * k : 2 * k + 2, ns],
                                    start=(k == 0), stop=(k == NKC // 2 - 1),
                                    perf_mode=DRQ,
                                )
                        nc.vector.tensor_scalar(
                            out=qT[:, m, :], in0=ps[:],
                            scalar1=cb[:, m : m + 1], scalar2=cb[:, 8:9],
                            op0=mybir.AluOpType.add, op1=mybir.AluOpType.mult,
                        )
                    # preload exp table after the last Sqrt (input depends
                    # on qT so the scheduler cannot hoist it before the LN)
                    nc.scalar.activation(
                        dummy[:], qT[:, 0, 0:1], mybir.ActivationFunctionType.Exp,
                    )

                # ---- k/v projections, context streamed in halves ----
                if True:
                    DR = mybir.MatmulPerfMode.DoubleRow
                    for ch in range(2):
                        hs = slice(ch * (TC // 2), (ch + 1) * (TC // 2))
                        # k projection for this context half (fp8 DoubleRow)
                        for m in range(NM):
                            ps = psmm.tile([P, TC // 2], FP, tag="mm")
                            for n in range(2):
                                ns = slice(ch * 1024 + n * 512, ch * 1024 + (n + 1) * 512)
                                for k in range(NKC // 2):
                                    nc.tensor.matmul(
                                        ps[:, n * 512 : (n + 1) * 512],
                                        wk[:, 2 * k : 2 * k + 2, m * P : (m + 1) * P],
                                        ctx8[:, 2 * k : 2 * k + 2, ns],
                                        start=(k == 0), stop=(k == NKC // 2 - 1),
                                        perf_mode=DR,
                                    )
                            nc.vector.tensor_scalar_add(
                                kTa[0:64, m, hs], ps[0:64, :], cb[0:64, 4 + m : 5 + m],
                            )
                            nc.vector.tensor_scalar_add(
                                kTb[64:128, m, hs], ps[64:128, :], cb[64:128, 4 + m : 5 + m],
                            )
                        # v projection for this context half (fp8 DoubleRow)
                        for jj in range(NJ // 2):
                            j = ch * (NJ // 2) + jj
                            ps = psmm.tile([P, CL], FP, tag="mmv")
                            for k in range(NKC // 2):
                                nc.tensor.matmul(
                                    ps[:],
                                    ctx8[:, 2 * k : 2 * k + 2, j * P : (j + 1) * P],
                                    wv[:, 2 * k : 2 * k + 2, :],
                                    start=(k == 0), stop=(k == NKC // 2 - 1),
                                    perf_mode=DR,
                                )
                            nc.vector.tensor_copy(
                                vv[:, j, :].rearrange("p (h c) -> p h c", c=HD + 1)[:, :, 0:HD],
                                ps[:].rearrange("p (h c) -> p h c", c=HD),
                            )

            # ---- attention: scores -> exp (ACT/DVE split) -> attn-out ----
            with (
                tc.tile_pool(name="expa", bufs=3) as eapool,
                tc.tile_pool(name="expb", bufs=2) as ebpool,
            ):
                with (
                    tc.tile_pool(name="pssc", bufs=4, space="PSUM") as pssc,
                    tc.tile_pool(name="psat", bufs=2, space="PSUM") as psat,
                ):
                    for i in range(NM):
                        ph = {}
                        for hh in range(2):
                            ph[hh] = psat.tile(
                                [HD + 1, T], FP, tag="ph", name=f"ph_{i}_{hh}",
                            )
                        ao_pending = []
                        for j in range(NJ):
                            js = slice(j * P, (j + 1) * P)
                            et_a = eapool.tile([P, T], BF, tag="eta", name=f"ea{i}_{j}")
                            et_b = ebpool.tile([P, T], I16, tag="etb", name=f"eb{i}_{j}")
                            for n in range(2):
                                ns = slice(n * 512, (n + 1) * 512)
                                psc_a = pssc.tile([P, 512], FP, tag="sc",
                                                  name=f"sa{i}_{j}_{n}")
                                psc_b = pssc.tile([P, 512], FP, tag="sc",
                                                  name=f"sb{i}_{j}_{n}")
                                nc.tensor.matmul(psc_a[:], kTa[:, i, js],
                                                 qT[:, i, ns],
                                                 start=True, stop=True)
                                nc.tensor.matmul(psc_b[:], kTb[:, i, js],
                                                 qT[:, i, ns],
                                                 start=True, stop=True)
                                nc.scalar.activation(
                                    et_a[:, ns], psc_a[:],
                                    mybir.ActivationFunctionType.Exp,
                                )
                                nc.vector.tensor_scalar(
                                    out=et_b[:, ns], in0=psc_b[:],
                                    scalar1=EXP_A, scalar2=EXP_B,
                                    op0=mybir.AluOpType.mult, op1=mybir.AluOpType.add,
                                )
                            ao_pending.append((j, et_a[:], et_b[:].bitcast(BF)))
                            if j >= 1:
                                jp, pa, pb = ao_pending.pop(0)
                                vs = vv[:, jp, :].rearrange("p (h c) -> p h c", c=HD + 1)
                                for n in range(2):
                                    ns = slice(n * 512, (n + 1) * 512)
                                    nc.tensor.matmul(
                                        ph[0][:, ns], vs[:, 2 * i, :], pa[:, ns],
                                        start=(jp == 0), stop=(jp == NJ - 1),
                                    )
                                    nc.tensor.matmul(
                                        ph[1][:, ns], vs[:, 2 * i + 1, :],
                                        pb[:, ns],
                                        start=(jp == 0), stop=(jp == NJ - 1),
                                    )
                        jp, pa, pb = ao_pending.pop(0)
                        vs = vv[:, jp, :].rearrange("p (h c) -> p h c", c=HD + 1)
                        for n in range(2):
                            ns = slice(n * 512, (n + 1) * 512)
                            nc.tensor.matmul(ph[0][:, ns], vs[:, 2 * i, :], pa[:, ns],
                                             start=(jp == 0), stop=(jp == NJ - 1))
                            nc.tensor.matmul(ph[1][:, ns], vs[:, 2 * i + 1, :],
                                             pb[:, ns],
                                             start=(jp == 0), stop=(jp == NJ - 1))
                        # evacuate: data rows -> attnU (ACT), denom rows -> denp (DVE)
                        for hh in range(2):
                            nc.scalar.copy(
                                attnU[64 * hh : 64 * hh + 64, i, :], ph[hh][0:64, :],
                            )
                            dp = 32 * (2 * (i % 2) + hh)
                            dc = (i // 2) * T
                            nc.vector.tensor_copy(
                                denp[dp : dp + 1, dc : dc + T], ph[hh][64:65, :],
                            )
                        if i == 1:
                            nc.vector.reciprocal_approx_fast(
                                recipp[:, 0:T], denp[:, 0:T],
                            )
                            nc.vector.tensor_copy(recipb[:, 0:T], recipp[:, 0:T])

                # ---- deferred normalize ----
                with tc.tile_pool(name="psel", bufs=2, space="PSUM") as psel:
                    nc.vector.reciprocal_approx_fast(
                        recipp[:, T : 2 * T], denp[:, T : 2 * T],
                    )
                    nc.vector.tensor_copy(recipb[:, T : 2 * T], recipp[:, T : 2 * T])
                    for i in range(NM):
                        dc = (i // 2) * T
                        rb = psel.tile([P, T], FP, tag="rb")
                        for n in range(2):
                            ns = slice(n * 512, (n + 1) * 512)
                            nc.tensor.matmul(
                                rb[:, ns], sel[:, i, :],
                                recipb[:, dc + n * 512 : dc + (n + 1) * 512],
                                start=True, stop=True,
                            )
                        nc.vector.tensor_mul(
                            attnT[:, i, :], attnU[:, i, :], rb[:],
                        )

                # ---- out-proj partials ----
                with tc.tile_pool(name="psoc", bufs=3, space="PSUM") as psoc:
                    with tc.tile_pool(name="oev", bufs=3) as opool:
                        for m in range(C // P):
                            po = psoc.tile([P, T], FP, tag="oc")
                            DRO = mybir.MatmulPerfMode.DoubleRow
                            for n in range(2):
                                ns = slice(n * 512, (n + 1) * 512)
                                for k2 in range(NM // 2):
                                    nc.tensor.matmul(
                                        po[:, ns],
                                        wo[:, 2 * k2 : 2 * k2 + 2, m * P : (m + 1) * P],
                                        attnT[:, 2 * k2 : 2 * k2 + 2, ns],
                                        start=(k2 == 0), stop=(k2 == NM // 2 - 1),
                                        perf_mode=DRO,
                                    )
                            ot = opool.tile([P, T], BF, tag="ot")
                            for n in range(2):
                                ns = slice(n * 512, (n + 1) * 512)
                                nc.vector.tensor_copy(ot[:, ns], po[:, ns])
                                nc.sync.dma_start(part_d[m][:, ns], ot[:, ns])

    nc.finalize()
    return nc


_NC_CACHE = {}


def _get_nc():
    if "nc" not in _NC_CACHE:
        _NC_CACHE["nc"] = _build_nc()
    return _NC_CACHE["nc"]


def _quant(w):
    g = np.float32(np.mean(np.abs(w), dtype=np.float64))
    t = np.clip(np.rint(w / (g + np.float32(Q_EPS))), -1.0, 1.0).astype(np.float32)
    return t, g


def _pack_kp(a):
    # [K, M] -> [P, K//P, M] (partition-major chunks)
    k, m = a.shape
    return np.ascontiguousarray(a.reshape(k // P, P, m).transpose(1, 0, 2))


def _bf(a):
    return np.ascontiguousarray(a.astype(ml_dtypes.bfloat16))


def _f8(a):
    return np.ascontiguousarray(a.astype(ml_dtypes.float8_e4m3))


def kernel(**inputs):
    global last_exec_time_ns
    x = np.asarray(inputs["x"], dtype=np.float32)
    ctx = np.asarray(inputs["context"], dtype=np.float32)
    Wq = np.asarray(inputs["Wq"], dtype=np.float32)
    Wk = np.asarray(inputs["Wk"], dtype=np.float32)
    Wv = np.asarray(inputs["Wv"], dtype=np.float32)
    Wo = np.asarray(inputs["Wo"], dtype=np.float32)
    bq = np.asarray(inputs["bq"], dtype=np.float32)
    bk = np.asarray(inputs["bk"], dtype=np.float32)
    bv = np.asarray(inputs["bv"], dtype=np.float32)
    bo = np.asarray(inputs["bo"], dtype=np.float32)
    g_ln = np.asarray(inputs["ln_gamma"], dtype=np.float32)
    b_ln = np.asarray(inputs["ln_beta"], dtype=np.float32)

    Tq, gq = _quant(Wq)
    Tk, gk = _quant(Wk)
    Tv, gv = _quant(Wv)
    To, go = _quant(Wo)

    qb_full = (bq + b_ln @ (gq * Tq).T) / gq          # [C]
    scale = np.float32(gq * gk * SCALE)
    host_bias = bo + bv @ (go * To).T                 # [C]

    # select matrices for the denominator broadcast: recipp partition
    # 32*(2*(i%2)+hh) feeds partitions [64*hh, 64*hh+64) of attnT chunk i
    selm = np.zeros((P, NM, P), dtype=np.float32)
    for i in range(NM):
        selm[32 * (2 * (i % 2)), i, 0:64] = 1.0
        selm[32 * (2 * (i % 2) + 1), i, 64:128] = 1.0

    in_maps = []
    for core in range(NCORES):
        b = core // 2
        g = core % 2
        rows = slice(CL * g, CL * (g + 1))
        wqT = _pack_kp((Tq[rows] * g_ln[None, :]).T)  # [P, 8, 512]
        wkT = _pack_kp(Tk[rows].T)
        wvT = _pack_kp(Tv[rows].T)
        woT = _pack_kp(To[:, rows].T)                 # [P, 4, 1024] ternary
        cbm = np.zeros((P, 9), dtype=np.float32)
        cbm[:, 0:4] = qb_full[rows].reshape(4, P).T
        cbm[:, 4:8] = (bk[rows] / gk).reshape(4, P).T
        cbm[:, 8] = scale
        in_maps.append({
            "x": _bf(x[b].reshape(T // P, P, C)),
            "ctxT": _f8(_pack_kp(np.ascontiguousarray(ctx[b].T))),
            "wqT": _f8(wqT), "wkT": _f8(wkT), "wvT": _f8(wvT), "woT": _f8(woT),
            "cb": cbm,
            "sel": _bf(selm),
        })

    nc = _get_nc()
    trace = os.environ.get("KERNEL_TRACE", "0") == "1"
    res = run_bass_kernel_spmd(nc, in_maps, list(range(NCORES)), trace=trace)
    last_exec_time_ns = res.exec_time_ns

    ogv = np.float32(go * gv)
    out = np.empty((B, T, C), dtype=np.float32)
    for b in range(B):
        p0 = res.results[2 * b]["partial"].astype(np.float32).reshape(C, T)
        p1 = res.results[2 * b + 1]["partial"].astype(np.float32).reshape(C, T)
        out[b] = x[b] + (p0.T + p1.T) * ogv + host_bias[None, :]
    return out


# revision 20
# speedup vs baseline: 1.2156x; 1.0317x over previous
"""Cross-modal attention block on 8 Trainium2 NeuronCores.

Sharding: core = 2*b + g  ->  batch b (4-way data parallel) x head-group g
(2-way tensor parallel over 16 heads -> 8 heads/core).  Each core:
  rownorm(x[b]) -> PE transpose -> q projection (ternary weights, gamma/beta
  folded) ; kT/v projections from pre-transposed context ; per-head
  scoresT = k~^T q~ ; exp split between ScalarE (exact) and VectorE
  (Schraudolph bit-trick into bf16) ; unnormalized attn-out with an appended
  ones-row producing softmax denominators in the same matmul ; deferred
  batch normalize (reciprocal_approx_fast + select-matmul broadcast) ;
  out-proj partial.  Host sums the two partials per batch + residual +
  folded biases.

All matmuls are full 128x128-mode bf16 (scores use zero-padded K so the PE
never enters a tiled mode, which measures as HAM-throttled 1.2 GHz).
"""

import os

import ml_dtypes
import numpy as np

import concourse.bass as bass
import concourse.mybir as mybir
import concourse.tile as tile
from concourse import bacc
from concourse.bass_utils import run_bass_kernel_spmd
from concourse.masks import make_identity

FP = mybir.dt.float32
FPR = mybir.dt.float32r
BF = mybir.dt.bfloat16
I16 = mybir.dt.int16
F8 = mybir.dt.float8e4

B, T, TC, C = 4, 1024, 2048, 1024
H, HD = 16, 64
HL = 8           # heads per core
CL = HL * HD     # 512 local channels
SCALE = HD ** -0.5
LN_EPS = 1e-5
Q_EPS = 1e-5
P = 128
NCORES = 8

NT = T // P      # 8 query-row tiles
NKC = C // P     # 8 contraction chunks over C
NJ = TC // P     # 16 context chunks
NM = CL // P     # 4 local d-chunks

# Schraudolph fast-exp into bf16 bit pattern via int16:
#   i16 = trunc(x * EXP_A + EXP_B); bf16 = bits(i16)
# max rel err ~3.3% over x in [-10, 8]; scores*scale stay well inside.
EXP_A = float(np.float32(128.0 / np.log(2.0)))
EXP_B = float(np.float32(16256.0 - 5.1))

last_exec_time_ns = None


def _build_nc():
    nc = bacc.Bacc(None, target_bir_lowering=False, debug=False)

    x_d = nc.dram_tensor("x", [NT, P, C], BF, kind="ExternalInput")
    ctxT_d = nc.dram_tensor("ctxT", [P, NKC, TC], F8, kind="ExternalInput")
    wqT_d = nc.dram_tensor("wqT", [P, NKC, CL], F8, kind="ExternalInput")
    wkT_d = nc.dram_tensor("wkT", [P, NKC, CL], F8, kind="ExternalInput")
    wvT_d = nc.dram_tensor("wvT", [P, NKC, CL], F8, kind="ExternalInput")
    woT_d = nc.dram_tensor("woT", [P, NM, C], F8, kind="ExternalInput")
    cb_d = nc.dram_tensor("cb", [P, 9], FP, kind="ExternalInput")
    sel_d = nc.dram_tensor("sel", [P, NM, P], BF, kind="ExternalInput")
    part_d = nc.dram_tensor("partial", [C // P, P, T], BF, kind="ExternalOutput")

    with tile.TileContext(nc) as tc:
        with (
            tc.tile_pool(name="const", bufs=1) as cpool,
            tc.tile_pool(name="acts", bufs=1) as apool,
        ):
            ident_f = cpool.tile([P, P], FP)
            make_identity(nc, ident_f[:])
            ident = cpool.tile([P, P], BF)
            nc.vector.tensor_copy(ident[:], ident_f[:])
            cb = cpool.tile([P, 9], FP)
            nc.sync.dma_start(cb[:], cb_d[:])
            sel = cpool.tile([P, NM, P], BF)
            nc.sync.dma_start(sel[:], sel_d[:])
            eps = cpool.tile([P, 1], FP)
            nc.vector.memset(eps[:], LN_EPS)

            wk = apool.tile([P, NKC, CL], F8, tag="wk")
            wv = apool.tile([P, NKC, CL], F8, tag="wv")
            wo = apool.tile([P, NM, C], F8, tag="wo")
            nc.sync.dma_start(wk[:], wkT_d[:])
            nc.sync.dma_start(wv[:], wvT_d[:])
            for k2 in range(NM):
                nc.sync.dma_start(wo[:, k2, :], woT_d[:, k2, :])
            rnT = apool.tile([P, NKC, T], F8, tag="rnT")
            # Scores: stationary kT holds both heads' rows; the moving side is
            # zero-padded per head (qTa rows 64-127 zero, qTb rows 0-63 zero)
            # so every scores matmul stays full-K 128x128 mode.
            qTa = apool.tile([P, NM, T], BF, tag="qTa")
            qTb = apool.tile([P, NM, T], BF, tag="qTb")
            kT = apool.tile([P, NM, TC], BF, tag="kT")
            vv = apool.tile([P, NJ, HL * (HD + 1)], BF, tag="vv")
            attnU = apool.tile([P, NM, T], BF, tag="attnU")
            attnT = apool.tile([P, NM, T], F8, tag="attnT")
            # denominator rows live at 32-aligned partitions (BIR requires
            # engine APs to start on partition multiples of 32):
            # row(i, hh) -> partition 32*(2*(i%2)+hh), column half i//2
            denp = apool.tile([P, 2 * T], FP, tag="denp")
            recipp = apool.tile([P, 2 * T], FP, tag="recipp")
            recipb = apool.tile([P, 2 * T], BF, tag="recipb")

            # one-time zero/one fills (DVE, overlapped with initial DMAs)
            nc.gpsimd.memset(qTa[64:128, :, :], 0.0)
            nc.gpsimd.memset(qTb[0:64, :, :], 0.0)
            nc.gpsimd.memset(denp[:], 1.0)
            # ones column of v' (denominator rows)
            nc.vector.memset(
                vv[:].rearrange("p j (h c) -> p (j h) c", c=HD + 1)[:, :, HD : HD + 1],
                1.0,
            )
            # preload the Exp activation table before the attention phase
            dummy = cpool.tile([P, 1], BF)

            with (
                tc.tile_pool(name="psmm", bufs=2, space="PSUM") as psmm,
                tc.tile_pool(name="ctx", bufs=1) as ctxpool,
            ):
                # ---- DMAs first; k/v projections lead while the LN
                # chains run on DVE/ACT underneath; transposes + q follow ----
                with (
                    tc.tile_pool(name="xrn", bufs=10) as xpool,
                    tc.tile_pool(name="xst", bufs=6) as spool,
                    tc.tile_pool(name="sqp", bufs=2) as sqpool,
                    tc.tile_pool(name="wqp", bufs=1) as wqpool,
                    tc.tile_pool(name="pstr", bufs=2, space="PSUM") as pstr,
                ):
                    xts = {}
                    for t in range(NT):
                        xts[t] = xpool.tile([P, C], BF, tag="xt", name=f"xt{t}")
                        nc.sync.dma_start(xts[t][:, 0:512], x_d[t][:, 0:512])
                        nc.sync.dma_start(xts[t][:, 512:1024], x_d[t][:, 512:1024])
                    wq = wqpool.tile([P, NKC, CL], F8, tag="wq")
                    nc.sync.dma_start(wq[:], wqT_d[:])
                    ctx8 = ctxpool.tile([P, NKC, TC], F8, tag="ctxT")
                    for k in range(NKC):
                        for hf in range(2):
                            nc.sync.dma_start(
                                ctx8[:, k, hf * 1024 : (hf + 1) * 1024],
                                ctxT_d[:, k, hf * 1024 : (hf + 1) * 1024],
                            )

                    rns = {}

                    def ln_chain(t):
                        xt = xts[t]
                        nmu = spool.tile([P, 1], FP, tag="nmu")
                        nc.vector.reduce_sum(nmu[:], xt[:], axis=mybir.AxisListType.X)
                        nc.scalar.mul(nmu[:], nmu[:], -1.0 / C)
                        sq = sqpool.tile([P, C], BF, tag="sq")
                        ex2 = spool.tile([P, 1], FP, tag="ex2")
                        nc.scalar.activation(
                            sq[:], xt[:], mybir.ActivationFunctionType.Square,
                            accum_out=ex2[:],
                        )
                        var = spool.tile([P, 1], FP, tag="var")
                        nc.scalar.mul(ex2[:], ex2[:], 1.0 / C)
                        mu2 = spool.tile([P, 1], FP, tag="mu2")
                        nc.vector.tensor_mul(mu2[:], nmu[:], nmu[:])
                        nc.vector.tensor_sub(var[:], ex2[:], mu2[:])
                        std = spool.tile([P, 1], FP, tag="std")
                        nc.scalar.activation(
                            std[:], var[:], mybir.ActivationFunctionType.Sqrt,
                            bias=eps[:],
                        )
                        inv = spool.tile([P, 1], FP, tag="inv")
                        nc.vector.reciprocal(inv[:], std[:])
                        rn = xpool.tile([P, C], BF, tag="rn", name=f"rn{t}")
                        nc.vector.scalar_tensor_tensor(
                            out=rn[:], in0=xt[:], scalar=nmu[:],
                            in1=inv[:].to_broadcast((P, C)),
                            op0=mybir.AluOpType.add, op1=mybir.AluOpType.mult,
                        )
                        rns[t] = rn

                    DR = mybir.MatmulPerfMode.DoubleRow
                    ln_t = 0
                    for ch in range(2):
                        hs = slice(ch * (TC // 2), (ch + 1) * (TC // 2))
                        # k projection for this context half (fp8 DoubleRow)
                        for m in range(NM):
                            ps = psmm.tile([P, TC // 2], FP, tag="mm")
                            for n in range(2):
                                ns = slice(ch * 1024 + n * 512, ch * 1024 + (n + 1) * 512)
                                for k in range(NKC // 2):
                                    nc.tensor.matmul(
                                        ps[:, n * 512 : (n + 1) * 512],
                                        wk[:, 2 * k : 2 * k + 2, m * P : (m + 1) * P],
                                        ctx8[:, 2 * k : 2 * k + 2, ns],
                                        start=(k == 0), stop=(k == NKC // 2 - 1),
                                        perf_mode=DR,
                                    )
                            nc.vector.tensor_scalar_add(
                                kT[:, m, hs], ps[:], cb[:, 4 + m : 5 + m],
                            )
                            if ln_t < NT:
                                ln_chain(ln_t)
                                ln_t += 1
                        # v projection for this context half (fp8 DoubleRow)
                        for jj in range(NJ // 2):
                            j = ch * (NJ // 2) + jj
                            ps = psmm.tile([P, CL], FP, tag="mmv")
                            for k in range(NKC // 2):
                                nc.tensor.matmul(
                                    ps[:],
                                    ctx8[:, 2 * k : 2 * k + 2, j * P : (j + 1) * P],
                                    wv[:, 2 * k : 2 * k + 2, :],
                                    start=(k == 0), stop=(k == NKC // 2 - 1),
                                    perf_mode=DR,
                                )
                            nc.vector.tensor_copy(
                                vv[:, j, :].rearrange("p (h c) -> p h c", c=HD + 1)[:, :, 0:HD],
                                ps[:].rearrange("p (h c) -> p h c", c=HD),
                            )
                            if ln_t < NT:
                                ln_chain(ln_t)
                                ln_t += 1

                    # ---- transposes (PE) after the k/v matmul stream ----
                    for t in range(NT):
                        rn = rns[t]
                        for c4 in range(2):
                            pt = pstr.tile([P, 512], BF, tag="ptr")
                            for cc in range(4):
                                c = 4 * c4 + cc
                                nc.tensor.transpose(
                                    pt[:, cc * P : (cc + 1) * P],
                                    rn[:, c * P : (c + 1) * P],
                                    ident[:],
                                )
                            nc.vector.tensor_copy(
                                rnT[:, 4 * c4 : 4 * c4 + 4, t * P : (t + 1) * P],
                                pt[:].rearrange("p (c q) -> p c q", q=P),
                            )

                    # ---- q projection (fp8 DoubleRow), zero-padded halves ----
                    for m in range(NM):
                        ps = psmm.tile([P, T], FP, tag="mm")
                        for n in range(2):
                            ns = slice(n * 512, (n + 1) * 512)
                            for k in range(NKC // 2):
                                nc.tensor.matmul(
                                    ps[:, ns],
                                    wq[:, 2 * k : 2 * k + 2, m * P : (m + 1) * P],
                                    rnT[:, 2 * k : 2 * k + 2, ns],
                                    start=(k == 0), stop=(k == NKC // 2 - 1),
                                    perf_mode=DR,
                                )
                        nc.vector.tensor_scalar(
                            out=qTa[0:64, m, :], in0=ps[0:64, :],
                            scalar1=cb[0:64, m : m + 1], scalar2=cb[0:64, 8:9],
                            op0=mybir.AluOpType.add, op1=mybir.AluOpType.mult,
                        )
                        nc.vector.tensor_scalar(
                            out=qTb[64:128, m, :], in0=ps[64:128, :],
                            scalar1=cb[64:128, m : m + 1], scalar2=cb[64:128, 8:9],
                            op0=mybir.AluOpType.add, op1=mybir.AluOpType.mult,
                        )
                    # preload exp table after the last Sqrt (input depends
                    # on qTa so the scheduler cannot hoist it before the LN)
                    nc.scalar.activation(
                        dummy[:], qTa[:, 0, 0:1], mybir.ActivationFunctionType.Exp,
                    )

            # ---- attention: scores -> exp (ACT/DVE split) -> attn-out ----
            with (
                tc.tile_pool(name="expa", bufs=3) as eapool,
                tc.tile_pool(name="expb", bufs=2) as ebpool,
            ):
                with (
                    tc.tile_pool(name="pssc", bufs=4, space="PSUM") as pssc,
                    tc.tile_pool(name="psat", bufs=2, space="PSUM") as psat,
                ):
                    for i in range(NM):
                        ph = {}
                        for hh in range(2):
                            ph[hh] = psat.tile(
                                [HD + 1, T], FP, tag="ph", name=f"ph_{i}_{hh}",
                            )
                        ao_pending = []
                        for j in range(NJ):
                            js = slice(j * P, (j + 1) * P)
                            et_a = eapool.tile([P, T], BF, tag="eta", name=f"ea{i}_{j}")
                            et_b = ebpool.tile([P, T], I16, tag="etb", name=f"eb{i}_{j}")
                            for n in range(2):
                                ns = slice(n * 512, (n + 1) * 512)
                                psc_a = pssc.tile([P, 512], FP, tag="sc",
                                                  name=f"sa{i}_{j}_{n}")
                                psc_b = pssc.tile([P, 512], FP, tag="sc",
                                                  name=f"sb{i}_{j}_{n}")
                                nc.tensor.matmul(psc_a[:], kT[:, i, js],
                                                 qTa[:, i, ns],
                                                 start=True, stop=True)
                                nc.tensor.matmul(psc_b[:], kT[:, i, js],
                                                 qTb[:, i, ns],
                                                 start=True, stop=True)
                                nc.scalar.activation(
                                    et_a[:, ns], psc_a[:],
                                    mybir.ActivationFunctionType.Exp,
                                )
                                nc.vector.tensor_scalar(
                                    out=et_b[:, ns], in0=psc_b[:],
                                    scalar1=EXP_A, scalar2=EXP_B,
                                    op0=mybir.AluOpType.mult, op1=mybir.AluOpType.add,
                                )
                            ao_pending.append((j, et_a[:], et_b[:].bitcast(BF)))
                            if j >= 1:
                                jp, pa, pb = ao_pending.pop(0)
                                vs = vv[:, jp, :].rearrange("p (h c) -> p h c", c=HD + 1)
                                for n in range(2):
                                    ns = slice(n * 512, (n + 1) * 512)
                                    nc.tensor.matmul(
                                        ph[0][:, ns], vs[:, 2 * i, :], pa[:, ns],
                                        start=(jp == 0), stop=(jp == NJ - 1),
                                    )
                                    nc.tensor.matmul(
                                        ph[1][:, ns], vs[:, 2 * i + 1, :],
                                        pb[:, ns],
                                        start=(jp == 0), stop=(jp == NJ - 1),
                                    )
                        jp, pa, pb = ao_pending.pop(0)
                        vs = vv[:, jp, :].rearrange("p (h c) -> p h c", c=HD + 1)
                        for n in range(2):
                            ns = slice(n * 512, (n + 1) * 512)
                            nc.tensor.matmul(ph[0][:, ns], vs[:, 2 * i, :], pa[:, ns],
                                             start=(jp == 0), stop=(jp == NJ - 1))
                            nc.tensor.matmul(ph[1][:, ns], vs[:, 2 * i + 1, :],
                                             pb[:, ns],
                                             start=(jp == 0), stop=(jp == NJ - 1))
                        # evacuate: data rows -> attnU (ACT), denom rows -> denp (DVE)
                        for hh in range(2):
                            nc.scalar.copy(
                                attnU[64 * hh : 64 * hh + 64, i, :], ph[hh][0:64, :],
                            )
                            dp = 32 * (2 * (i % 2) + hh)
                            dc = (i // 2) * T
                            nc.vector.tensor_copy(
                                denp[dp : dp + 1, dc : dc + T], ph[hh][64:65, :],
                            )
                        if i == 1:
                            nc.vector.reciprocal_approx_fast(
                                recipp[:, 0:T], denp[:, 0:T],
                            )
                            nc.vector.tensor_copy(recipb[:, 0:T], recipp[:, 0:T])

                # ---- deferred normalize ----
                with tc.tile_pool(name="psel", bufs=2, space="PSUM") as psel:
                    nc.vector.reciprocal_approx_fast(
                        recipp[:, T : 2 * T], denp[:, T : 2 * T],
                    )
                    nc.vector.tensor_copy(recipb[:, T : 2 * T], recipp[:, T : 2 * T])
                    for i in range(NM):
                        dc = (i // 2) * T
                        rb = psel.tile([P, T], FP, tag="rb")
                        for n in range(2):
                            ns = slice(n * 512, (n + 1) * 512)
                            nc.tensor.matmul(
                                rb[:, ns], sel[:, i, :],
                                recipb[:, dc + n * 512 : dc + (n + 1) * 512],
                                start=True, stop=True,
                            )
                        nc.vector.tensor_mul(
                            attnT[:, i, :], attnU[:, i, :], rb[:],
                        )

                # ---- out-proj partials ----
                with tc.tile_pool(name="psoc", bufs=3, space="PSUM") as psoc:
                    with tc.tile_pool(name="oev", bufs=3) as opool:
                        for m in range(C // P):
                            po = psoc.tile([P, T], FP, tag="oc")
                            DRO = mybir.MatmulPerfMode.DoubleRow
                            for n in range(2):
                                ns = slice(n * 512, (n + 1) * 512)
                                for k2 in range(NM // 2):
                                    nc.tensor.matmul(
                                        po[:, ns],
                                        wo[:, 2 * k2 : 2 * k2 + 2, m * P : (m + 1) * P],
                                        attnT[:, 2 * k2 : 2 * k2 + 2, ns],
                                        start=(k2 == 0), stop=(k2 == NM // 2 - 1),
                                        perf_mode=DRO,
                                    )
                            ot = opool.tile([P, T], BF, tag="ot")
                            for n in range(2):
                                ns = slice(n * 512, (n + 1) * 512)
                                nc.vector.tensor_copy(ot[:, ns], po[:, ns])
                                nc.sync.dma_start(part_d[m][:, ns], ot[:, ns])

    nc.finalize()
    return nc


_NC_CACHE = {}


def _get_nc():
    if "nc" not in _NC_CACHE:
        _NC_CACHE["nc"] = _build_nc()
    return _NC_CACHE["nc"]


def _quant(w):
    g = np.float32(np.mean(np.abs(w), dtype=np.float64))
    t = np.clip(np.rint(w / (g + np.float32(Q_EPS))), -1.0, 1.0).astype(np.float32)
    return t, g


def _pack_kp(a):
    # [K, M] -> [P, K//P, M] (partition-major chunks)
    k, m = a.shape
    return np.ascontiguousarray(a.reshape(k // P, P, m).transpose(1, 0, 2))


def _bf(a):
    return np.ascontiguousarray(a.astype(ml_dtypes.bfloat16))


def _f8(a):
    return np.ascontiguousarray(a.astype(ml_dtypes.float8_e4m3))


def kernel(**inputs):
    global last_exec_time_ns
    x = np.asarray(inputs["x"], dtype=np.float32)
    ctx = np.asarray(inputs["context"], dtype=np.float32)
    Wq = np.asarray(inputs["Wq"], dtype=np.float32)
    Wk = np.asarray(inputs["Wk"], dtype=np.float32)
    Wv = np.asarray(inputs["Wv"], dtype=np.float32)
    Wo = np.asarray(inputs["Wo"], dtype=np.float32)
    bq = np.asarray(inputs["bq"], dtype=np.float32)
    bk = np.asarray(inputs["bk"], dtype=np.float32)
    bv = np.asarray(inputs["bv"], dtype=np.float32)
    bo = np.asarray(inputs["bo"], dtype=np.float32)
    g_ln = np.asarray(inputs["ln_gamma"], dtype=np.float32)
    b_ln = np.asarray(inputs["ln_beta"], dtype=np.float32)

    Tq, gq = _quant(Wq)
    Tk, gk = _quant(Wk)
    Tv, gv = _quant(Wv)
    To, go = _quant(Wo)

    qb_full = (bq + b_ln @ (gq * Tq).T) / gq          # [C]
    scale = np.float32(gq * gk * SCALE)
    host_bias = bo + bv @ (go * To).T                 # [C]

    # select matrices for the denominator broadcast: recipp partition
    # 32*(2*(i%2)+hh) feeds partitions [64*hh, 64*hh+64) of attnT chunk i
    selm = np.zeros((P, NM, P), dtype=np.float32)
    for i in range(NM):
        selm[32 * (2 * (i % 2)), i, 0:64] = 1.0
        selm[32 * (2 * (i % 2) + 1), i, 64:128] = 1.0

    in_maps = []
    for core in range(NCORES):
        b = core // 2
        g = core % 2
        rows = slice(CL * g, CL * (g + 1))
        wqT = _pack_kp((Tq[rows] * g_ln[None, :]).T)  # [P, 8, 512]
        wkT = _pack_kp(Tk[rows].T)
        wvT = _pack_kp(Tv[rows].T)
        woT = _pack_kp(To[:, rows].T)                 # [P, 4, 1024] ternary
        cbm = np.zeros((P, 9), dtype=np.float32)
        cbm[:, 0:4] = qb_full[rows].reshape(4, P).T
        cbm[:, 4:8] = (bk[rows] / gk).reshape(4, P).T
        cbm[:, 8] = scale
        in_maps.append({
            "x": _bf(x[b].reshape(T // P, P, C)),
            "ctxT": _f8(_pack_kp(np.ascontiguousarray(ctx[b].T))),
            "wqT": _f8(wqT), "wkT": _f8(wkT), "wvT": _f8(wvT), "woT": _f8(woT),
            "cb": cbm,
            "sel": _bf(selm),
        })

    nc = _get_nc()
    trace = os.environ.get("KERNEL_TRACE", "0") == "1"
    res = run_bass_kernel_spmd(nc, in_maps, list(range(NCORES)), trace=trace)
    last_exec_time_ns = res.exec_time_ns

    ogv = np.float32(go * gv)
    out = np.empty((B, T, C), dtype=np.float32)
    for b in range(B):
        p0 = res.results[2 * b]["partial"].astype(np.float32).reshape(C, T)
        p1 = res.results[2 * b + 1]["partial"].astype(np.float32).reshape(C, T)
        out[b] = x[b] + (p0.T + p1.T) * ogv + host_bias[None, :]
    return out


# revision 21
# speedup vs baseline: 1.2666x; 1.0420x over previous
"""Cross-modal attention block on 8 Trainium2 NeuronCores.

Sharding: core = 2*b + g  ->  batch b (4-way data parallel) x head-group g
(2-way tensor parallel over 16 heads -> 8 heads/core).  Each core:
  rownorm(x[b]) -> PE transpose -> q projection (ternary weights, gamma/beta
  folded) ; kT/v projections from pre-transposed context ; per-head
  scoresT = k~^T q~ ; exp split between ScalarE (exact) and VectorE
  (Schraudolph bit-trick into bf16) ; unnormalized attn-out with an appended
  ones-row producing softmax denominators in the same matmul ; deferred
  batch normalize (reciprocal_approx_fast + select-matmul broadcast) ;
  out-proj partial.  Host sums the two partials per batch + residual +
  folded biases.

All matmuls are full 128x128-mode bf16 (scores use zero-padded K so the PE
never enters a tiled mode, which measures as HAM-throttled 1.2 GHz).
"""

import os

import ml_dtypes
import numpy as np

import concourse.bass as bass
import concourse.mybir as mybir
import concourse.tile as tile
from concourse import bacc
from concourse.bass_utils import run_bass_kernel_spmd
from concourse.masks import make_identity

FP = mybir.dt.float32
FPR = mybir.dt.float32r
BF = mybir.dt.bfloat16
I16 = mybir.dt.int16
F8 = mybir.dt.float8e4

B, T, TC, C = 4, 1024, 2048, 1024
H, HD = 16, 64
HL = 8           # heads per core
CL = HL * HD     # 512 local channels
SCALE = HD ** -0.5
LN_EPS = 1e-5
Q_EPS = 1e-5
P = 128
NCORES = 8

NT = T // P      # 8 query-row tiles
NKC = C // P     # 8 contraction chunks over C
NJ = TC // P     # 16 context chunks
NM = CL // P     # 4 local d-chunks

# Schraudolph fast-exp into bf16 bit pattern via int16:
#   i16 = trunc(x * EXP_A + EXP_B); bf16 = bits(i16)
# max rel err ~3.3% over x in [-10, 8]; scores*scale stay well inside.
EXP_A = float(np.float32(128.0 / np.log(2.0)))
EXP_B = float(np.float32(16256.0 - 5.1))

last_exec_time_ns = None


def _build_nc():
    nc = bacc.Bacc(None, target_bir_lowering=False, debug=False)

    x_d = nc.dram_tensor("x", [NT, P, C], BF, kind="ExternalInput")
    ctxT_d = nc.dram_tensor("ctxT", [P, NKC, TC], F8, kind="ExternalInput")
    wqT_d = nc.dram_tensor("wqT", [P, NKC, CL], F8, kind="ExternalInput")
    wkT_d = nc.dram_tensor("wkT", [P, NKC, CL], F8, kind="ExternalInput")
    wvT_d = nc.dram_tensor("wvT", [P, NKC, CL], F8, kind="ExternalInput")
    woT_d = nc.dram_tensor("woT", [P, NM, C], F8, kind="ExternalInput")
    cb_d = nc.dram_tensor("cb", [P, 9], FP, kind="ExternalInput")
    sel_d = nc.dram_tensor("sel", [P, NM, P], BF, kind="ExternalInput")
    part_d = nc.dram_tensor("partial", [C // P, P, T], BF, kind="ExternalOutput")

    with tile.TileContext(nc) as tc:
        with (
            tc.tile_pool(name="const", bufs=1) as cpool,
            tc.tile_pool(name="acts", bufs=1) as apool,
        ):
            ident_f = cpool.tile([P, P], FP)
            make_identity(nc, ident_f[:])
            ident = cpool.tile([P, P], BF)
            nc.vector.tensor_copy(ident[:], ident_f[:])
            cb = cpool.tile([P, 9], FP)
            nc.sync.dma_start(cb[:], cb_d[:])
            sel = cpool.tile([P, NM, P], BF)
            nc.sync.dma_start(sel[:], sel_d[:])
            eps = cpool.tile([P, 1], FP)
            nc.vector.memset(eps[:], LN_EPS)

            wk = apool.tile([P, NKC, CL], F8, tag="wk")
            wv = apool.tile([P, NKC, CL], F8, tag="wv")
            wo = apool.tile([P, NM, C], F8, tag="wo")
            nc.sync.dma_start(wk[:], wkT_d[:])
            nc.sync.dma_start(wv[:], wvT_d[:])
            for k2 in range(NM):
                nc.sync.dma_start(wo[:, k2, :], woT_d[:, k2, :])
            rnT = apool.tile([P, NKC, T], F8, tag="rnT")
            # Scores: stationary kT holds both heads' rows; the moving side is
            # zero-padded per head (qTa rows 64-127 zero, qTb rows 0-63 zero)
            # so every scores matmul stays full-K 128x128 mode.
            qTa = apool.tile([P, NM, T], BF, tag="qTa")
            qTb = apool.tile([P, NM, T], BF, tag="qTb")
            kT = apool.tile([P, NM, TC], BF, tag="kT")
            vv = apool.tile([P, NJ, HL * (HD + 1)], BF, tag="vv")
            attnU = apool.tile([P, NM, T], BF, tag="attnU")
            attnT = apool.tile([P, NM, T], F8, tag="attnT")
            # denominator rows live at 32-aligned partitions (BIR requires
            # engine APs to start on partition multiples of 32):
            # row(i, hh) -> partition 32*(2*(i%2)+hh), column half i//2
            denp = apool.tile([P, 2 * T], FP, tag="denp")
            recipp = apool.tile([P, 2 * T], FP, tag="recipp")
            recipb = apool.tile([P, 2 * T], BF, tag="recipb")

            # one-time zero/one fills (DVE, overlapped with initial DMAs)
            nc.gpsimd.memset(qTa[64:128, :, :], 0.0)
            nc.gpsimd.memset(qTb[0:64, :, :], 0.0)
            nc.gpsimd.memset(denp[:], 1.0)
            # ones column of v' (denominator rows)
            nc.vector.memset(
                vv[:].rearrange("p j (h c) -> p (j h) c", c=HD + 1)[:, :, HD : HD + 1],
                1.0,
            )
            # preload the Exp activation table before the attention phase
            dummy = cpool.tile([P, 1], BF)

            with (
                tc.tile_pool(name="psmm", bufs=2, space="PSUM") as psmm,
                tc.tile_pool(name="ctx", bufs=1) as ctxpool,
            ):
                # ---- DMAs first; k/v projections lead while the LN
                # chains run on DVE/ACT underneath; transposes + q follow ----
                with (
                    tc.tile_pool(name="xrn", bufs=10) as xpool,
                    tc.tile_pool(name="xst", bufs=6) as spool,
                    tc.tile_pool(name="sqp", bufs=2) as sqpool,
                    tc.tile_pool(name="wqp", bufs=1) as wqpool,
                    tc.tile_pool(name="pstr", bufs=2, space="PSUM") as pstr,
                ):
                    xts = {}
                    for t in range(NT):
                        xts[t] = xpool.tile([P, C], BF, tag="xt", name=f"xt{t}")
                        nc.sync.dma_start(xts[t][:, 0:512], x_d[t][:, 0:512])
                        nc.sync.dma_start(xts[t][:, 512:1024], x_d[t][:, 512:1024])
                    wq = wqpool.tile([P, NKC, CL], F8, tag="wq")
                    nc.sync.dma_start(wq[:], wqT_d[:])
                    ctx8 = ctxpool.tile([P, NKC, TC], F8, tag="ctxT")
                    for k in range(NKC):
                        for hf in range(2):
                            nc.sync.dma_start(
                                ctx8[:, k, hf * 1024 : (hf + 1) * 1024],
                                ctxT_d[:, k, hf * 1024 : (hf + 1) * 1024],
                            )

                    rns = {}

                    def ln_chain(t):
                        xt = xts[t]
                        nmu = spool.tile([P, 1], FP, tag="nmu")
                        nc.vector.reduce_sum(nmu[:], xt[:], axis=mybir.AxisListType.X)
                        nc.scalar.mul(nmu[:], nmu[:], -1.0 / C)
                        sq = sqpool.tile([P, C], BF, tag="sq")
                        ex2 = spool.tile([P, 1], FP, tag="ex2")
                        nc.scalar.activation(
                            sq[:], xt[:], mybir.ActivationFunctionType.Square,
                            accum_out=ex2[:],
                        )
                        var = spool.tile([P, 1], FP, tag="var")
                        nc.scalar.mul(ex2[:], ex2[:], 1.0 / C)
                        mu2 = spool.tile([P, 1], FP, tag="mu2")
                        nc.vector.tensor_mul(mu2[:], nmu[:], nmu[:])
                        nc.vector.tensor_sub(var[:], ex2[:], mu2[:])
                        std = spool.tile([P, 1], FP, tag="std")
                        nc.scalar.activation(
                            std[:], var[:], mybir.ActivationFunctionType.Sqrt,
                            bias=eps[:],
                        )
                        inv = spool.tile([P, 1], FP, tag="inv")
                        nc.vector.reciprocal(inv[:], std[:])
                        rn = xpool.tile([P, C], BF, tag="rn", name=f"rn{t}")
                        nc.vector.scalar_tensor_tensor(
                            out=rn[:], in0=xt[:], scalar=nmu[:],
                            in1=inv[:].to_broadcast((P, C)),
                            op0=mybir.AluOpType.add, op1=mybir.AluOpType.mult,
                        )
                        rns[t] = rn

                    DR = mybir.MatmulPerfMode.DoubleRow
                    ln_t = 0
                    for ch in range(2):
                        hs = slice(ch * (TC // 2), (ch + 1) * (TC // 2))
                        # k projection for this context half (fp8 DoubleRow)
                        for m in range(NM):
                            ps = psmm.tile([P, TC // 2], FP, tag="mm")
                            for n in range(2):
                                ns = slice(ch * 1024 + n * 512, ch * 1024 + (n + 1) * 512)
                                for k in range(NKC // 2):
                                    nc.tensor.matmul(
                                        ps[:, n * 512 : (n + 1) * 512],
                                        wk[:, 2 * k : 2 * k + 2, m * P : (m + 1) * P],
                                        ctx8[:, 2 * k : 2 * k + 2, ns],
                                        start=(k == 0), stop=(k == NKC // 2 - 1),
                                        perf_mode=DR,
                                    )
                            nc.vector.tensor_scalar_add(
                                kT[:, m, hs], ps[:], cb[:, 4 + m : 5 + m],
                            )
                            if ln_t < NT:
                                ln_chain(ln_t)
                                ln_t += 1
                        # v projection for this context half (fp8 DoubleRow)
                        for jj in range(NJ // 2):
                            j = ch * (NJ // 2) + jj
                            ps = psmm.tile([P, CL], FP, tag="mmv")
                            for k in range(NKC // 2):
                                nc.tensor.matmul(
                                    ps[:],
                                    ctx8[:, 2 * k : 2 * k + 2, j * P : (j + 1) * P],
                                    wv[:, 2 * k : 2 * k + 2, :],
                                    start=(k == 0), stop=(k == NKC // 2 - 1),
                                    perf_mode=DR,
                                )
                            nc.vector.tensor_copy(
                                vv[:, j, :].rearrange("p (h c) -> p h c", c=HD + 1)[:, :, 0:HD],
                                ps[:].rearrange("p (h c) -> p h c", c=HD),
                            )
                            if ln_t < NT:
                                ln_chain(ln_t)
                                ln_t += 1

                    # ---- transposes (PE) after the k/v matmul stream ----
                    for t in range(NT):
                        rn = rns[t]
                        for c4 in range(2):
                            pt = pstr.tile([P, 512], BF, tag="ptr")
                            for cc in range(4):
                                c = 4 * c4 + cc
                                nc.tensor.transpose(
                                    pt[:, cc * P : (cc + 1) * P],
                                    rn[:, c * P : (c + 1) * P],
                                    ident[:],
                                )
                            nc.scalar.copy(
                                rnT[:, 4 * c4 : 4 * c4 + 4, t * P : (t + 1) * P],
                                pt[:].rearrange("p (c q) -> p c q", q=P),
                            )

                    # ---- q projection (fp8 DoubleRow), zero-padded halves ----
                    for m in range(NM):
                        ps = psmm.tile([P, T], FP, tag="mm")
                        for n in range(2):
                            ns = slice(n * 512, (n + 1) * 512)
                            for k in range(NKC // 2):
                                nc.tensor.matmul(
                                    ps[:, ns],
                                    wq[:, 2 * k : 2 * k + 2, m * P : (m + 1) * P],
                                    rnT[:, 2 * k : 2 * k + 2, ns],
                                    start=(k == 0), stop=(k == NKC // 2 - 1),
                                    perf_mode=DR,
                                )
                        nc.vector.tensor_scalar(
                            out=qTa[0:64, m, :], in0=ps[0:64, :],
                            scalar1=cb[0:64, m : m + 1], scalar2=cb[0:64, 8:9],
                            op0=mybir.AluOpType.add, op1=mybir.AluOpType.mult,
                        )
                        nc.vector.tensor_scalar(
                            out=qTb[64:128, m, :], in0=ps[64:128, :],
                            scalar1=cb[64:128, m : m + 1], scalar2=cb[64:128, 8:9],
                            op0=mybir.AluOpType.add, op1=mybir.AluOpType.mult,
                        )
                    # preload exp table after the last Sqrt (input depends
                    # on qTa so the scheduler cannot hoist it before the LN)
                    nc.scalar.activation(
                        dummy[:], qTa[:, 0, 0:1], mybir.ActivationFunctionType.Exp,
                    )

            # ---- attention: scores -> exp (ACT/DVE split) -> attn-out ----
            with (
                tc.tile_pool(name="expa", bufs=3) as eapool,
                tc.tile_pool(name="expb", bufs=2) as ebpool,
            ):
                with (
                    tc.tile_pool(name="pssc", bufs=4, space="PSUM") as pssc,
                    tc.tile_pool(name="psat", bufs=2, space="PSUM") as psat,
                ):
                    for i in range(NM):
                        ph = {}
                        for hh in range(2):
                            ph[hh] = psat.tile(
                                [HD + 1, T], FP, tag="ph", name=f"ph_{i}_{hh}",
                            )
                        ao_pending = []
                        for j in range(NJ):
                            js = slice(j * P, (j + 1) * P)
                            et_a = eapool.tile([P, T], BF, tag="eta", name=f"ea{i}_{j}")
                            et_b = ebpool.tile([P, T], I16, tag="etb", name=f"eb{i}_{j}")
                            for n in range(2):
                                ns = slice(n * 512, (n + 1) * 512)
                                psc_a = pssc.tile([P, 512], FP, tag="sc",
                                                  name=f"sa{i}_{j}_{n}")
                                psc_b = pssc.tile([P, 512], FP, tag="sc",
                                                  name=f"sb{i}_{j}_{n}")
                                nc.tensor.matmul(psc_a[:], kT[:, i, js],
                                                 qTa[:, i, ns],
                                                 start=True, stop=True)
                                nc.tensor.matmul(psc_b[:], kT[:, i, js],
                                                 qTb[:, i, ns],
                                                 start=True, stop=True)
                                nc.scalar.activation(
                                    et_a[:, ns], psc_a[:],
                                    mybir.ActivationFunctionType.Exp,
                                )
                                nc.vector.tensor_scalar(
                                    out=et_b[:, ns], in0=psc_b[:],
                                    scalar1=EXP_A, scalar2=EXP_B,
                                    op0=mybir.AluOpType.mult, op1=mybir.AluOpType.add,
                                )
                            ao_pending.append((j, et_a[:], et_b[:].bitcast(BF)))
                            if j >= 1:
                                jp, pa, pb = ao_pending.pop(0)
                                vs = vv[:, jp, :].rearrange("p (h c) -> p h c", c=HD + 1)
                                for n in range(2):
                                    ns = slice(n * 512, (n + 1) * 512)
                                    nc.tensor.matmul(
                                        ph[0][:, ns], vs[:, 2 * i, :], pa[:, ns],
                                        start=(jp == 0), stop=(jp == NJ - 1),
                                    )
                                    nc.tensor.matmul(
                                        ph[1][:, ns], vs[:, 2 * i + 1, :],
                                        pb[:, ns],
                                        start=(jp == 0), stop=(jp == NJ - 1),
                                    )
                        jp, pa, pb = ao_pending.pop(0)
                        vs = vv[:, jp, :].rearrange("p (h c) -> p h c", c=HD + 1)
                        for n in range(2):
                            ns = slice(n * 512, (n + 1) * 512)
                            nc.tensor.matmul(ph[0][:, ns], vs[:, 2 * i, :], pa[:, ns],
                                             start=(jp == 0), stop=(jp == NJ - 1))
                            nc.tensor.matmul(ph[1][:, ns], vs[:, 2 * i + 1, :],
                                             pb[:, ns],
                                             start=(jp == 0), stop=(jp == NJ - 1))
                        # evacuate: data rows -> attnU (ACT), denom rows -> denp (DVE)
                        for hh in range(2):
                            nc.scalar.copy(
                                attnU[64 * hh : 64 * hh + 64, i, :], ph[hh][0:64, :],
                            )
                            dp = 32 * (2 * (i % 2) + hh)
                            dc = (i // 2) * T
                            nc.vector.tensor_copy(
                                denp[dp : dp + 1, dc : dc + T], ph[hh][64:65, :],
                            )
                        if i == 1:
                            nc.vector.reciprocal_approx_fast(
                                recipp[:, 0:T], denp[:, 0:T],
                            )
                            nc.vector.tensor_copy(recipb[:, 0:T], recipp[:, 0:T])

                # ---- deferred normalize ----
                with tc.tile_pool(name="psel", bufs=2, space="PSUM") as psel:
                    nc.vector.reciprocal_approx_fast(
                        recipp[:, T : 2 * T], denp[:, T : 2 * T],
                    )
                    nc.vector.tensor_copy(recipb[:, T : 2 * T], recipp[:, T : 2 * T])
                    for i in range(NM):
                        dc = (i // 2) * T
                        rb = psel.tile([P, T], FP, tag="rb")
                        for n in range(2):
                            ns = slice(n * 512, (n + 1) * 512)
                            nc.tensor.matmul(
                                rb[:, ns], sel[:, i, :],
                                recipb[:, dc + n * 512 : dc + (n + 1) * 512],
                                start=True, stop=True,
                            )
                        nc.vector.tensor_mul(
                            attnT[:, i, :], attnU[:, i, :], rb[:],
                        )

                # ---- out-proj partials ----
                with tc.tile_pool(name="psoc", bufs=3, space="PSUM") as psoc:
                    with tc.tile_pool(name="oev", bufs=3) as opool:
                        for m in range(C // P):
                            po = psoc.tile([P, T], FP, tag="oc")
                            DRO = mybir.MatmulPerfMode.DoubleRow
                            for n in range(2):
                                ns = slice(n * 512, (n + 1) * 512)
                                for k2 in range(NM // 2):
                                    nc.tensor.matmul(
                                        po[:, ns],
                                        wo[:, 2 * k2 : 2 * k2 + 2, m * P : (m + 1) * P],
                                        attnT[:, 2 * k2 : 2 * k2 + 2, ns],
                                        start=(k2 == 0), stop=(k2 == NM // 2 - 1),
                                        perf_mode=DRO,
                                    )
                            ot = opool.tile([P, T], BF, tag="ot")
                            for n in range(2):
                                ns = slice(n * 512, (n + 1) * 512)
                                nc.scalar.copy(ot[:, ns], po[:, ns])
                                nc.sync.dma_start(part_d[m][:, ns], ot[:, ns])

    nc.finalize()
    return nc


_NC_CACHE = {}


def _get_nc():
    if "nc" not in _NC_CACHE:
        _NC_CACHE["nc"] = _build_nc()
    return _NC_CACHE["nc"]


def _quant(w):
    g = np.float32(np.mean(np.abs(w), dtype=np.float64))
    t = np.clip(np.rint(w / (g + np.float32(Q_EPS))), -1.0, 1.0).astype(np.float32)
    return t, g


def _pack_kp(a):
    # [K, M] -> [P, K//P, M] (partition-major chunks)
    k, m = a.shape
    return np.ascontiguousarray(a.reshape(k // P, P, m).transpose(1, 0, 2))


def _bf(a):
    return np.ascontiguousarray(a.astype(ml_dtypes.bfloat16))


def _f8(a):
    return np.ascontiguousarray(a.astype(ml_dtypes.float8_e4m3))


def kernel(**inputs):
    global last_exec_time_ns
    x = np.asarray(inputs["x"], dtype=np.float32)
    ctx = np.asarray(inputs["context"], dtype=np.float32)
    Wq = np.asarray(inputs["Wq"], dtype=np.float32)
    Wk = np.asarray(inputs["Wk"], dtype=np.float32)
    Wv = np.asarray(inputs["Wv"], dtype=np.float32)
    Wo = np.asarray(inputs["Wo"], dtype=np.float32)
    bq = np.asarray(inputs["bq"], dtype=np.float32)
    bk = np.asarray(inputs["bk"], dtype=np.float32)
    bv = np.asarray(inputs["bv"], dtype=np.float32)
    bo = np.asarray(inputs["bo"], dtype=np.float32)
    g_ln = np.asarray(inputs["ln_gamma"], dtype=np.float32)
    b_ln = np.asarray(inputs["ln_beta"], dtype=np.float32)

    Tq, gq = _quant(Wq)
    Tk, gk = _quant(Wk)
    Tv, gv = _quant(Wv)
    To, go = _quant(Wo)

    qb_full = (bq + b_ln @ (gq * Tq).T) / gq          # [C]
    scale = np.float32(gq * gk * SCALE)
    host_bias = bo + bv @ (go * To).T                 # [C]

    # select matrices for the denominator broadcast: recipp partition
    # 32*(2*(i%2)+hh) feeds partitions [64*hh, 64*hh+64) of attnT chunk i
    selm = np.zeros((P, NM, P), dtype=np.float32)
    for i in range(NM):
        selm[32 * (2 * (i % 2)), i, 0:64] = 1.0
        selm[32 * (2 * (i % 2) + 1), i, 64:128] = 1.0

    in_maps = []
    for core in range(NCORES):
        b = core // 2
        g = core % 2
        rows = slice(CL * g, CL * (g + 1))
        wqT = _pack_kp((Tq[rows] * g_ln[None, :]).T)  # [P, 8, 512]
        wkT = _pack_kp(Tk[rows].T)
        wvT = _pack_kp(Tv[rows].T)
        woT = _pack_kp(To[:, rows].T)                 # [P, 4, 1024] ternary
        cbm = np.zeros((P, 9), dtype=np.float32)
        cbm[:, 0:4] = qb_full[rows].reshape(4, P).T
        cbm[:, 4:8] = (bk[rows] / gk).reshape(4, P).T
        cbm[:, 8] = scale
        in_maps.append({
            "x": _bf(x[b].reshape(T // P, P, C)),
            "ctxT": _f8(_pack_kp(np.ascontiguousarray(ctx[b].T))),
            "wqT": _f8(wqT), "wkT": _f8(wkT), "wvT": _f8(wvT), "woT": _f8(woT),
            "cb": cbm,
            "sel": _bf(selm),
        })

    nc = _get_nc()
    trace = os.environ.get("KERNEL_TRACE", "0") == "1"
    res = run_bass_kernel_spmd(nc, in_maps, list(range(NCORES)), trace=trace)
    last_exec_time_ns = res.exec_time_ns

    ogv = np.float32(go * gv)
    out = np.empty((B, T, C), dtype=np.float32)
    for b in range(B):
        p0 = res.results[2 * b]["partial"].astype(np.float32).reshape(C, T)
        p1 = res.results[2 * b + 1]["partial"].astype(np.float32).reshape(C, T)
        out[b] = x[b] + (p0.T + p1.T) * ogv + host_bias[None, :]
    return out


# revision 24
# speedup vs baseline: 1.3265x; 1.0473x over previous
"""Cross-modal attention block on 8 Trainium2 NeuronCores.

Sharding: core = 2*b + g  ->  batch b (4-way data parallel) x head-group g
(2-way tensor parallel over 16 heads -> 8 heads/core).  Each core:
  rownorm(x[b]) -> PE transpose -> q projection (ternary weights, gamma/beta
  folded) ; kT/v projections from pre-transposed context ; per-head
  scoresT = k~^T q~ ; exp split between ScalarE (exact) and VectorE
  (Schraudolph bit-trick into bf16) ; unnormalized attn-out with an appended
  ones-row producing softmax denominators in the same matmul ; deferred
  batch normalize (reciprocal_approx_fast + select-matmul broadcast) ;
  out-proj partial.  Host sums the two partials per batch + residual +
  folded biases.

All matmuls are full 128x128-mode bf16 (scores use zero-padded K so the PE
never enters a tiled mode, which measures as HAM-throttled 1.2 GHz).
"""

import os

import ml_dtypes
import numpy as np

import concourse.bass as bass
import concourse.mybir as mybir
import concourse.tile as tile
from concourse import bacc
from concourse.bass_utils import run_bass_kernel_spmd
from concourse.masks import make_identity

FP = mybir.dt.float32
FPR = mybir.dt.float32r
BF = mybir.dt.bfloat16
I16 = mybir.dt.int16
F8 = mybir.dt.float8e4

B, T, TC, C = 4, 1024, 2048, 1024
H, HD = 16, 64
HL = 8           # heads per core
CL = HL * HD     # 512 local channels
SCALE = HD ** -0.5
LN_EPS = 1e-5
Q_EPS = 1e-5
P = 128
NCORES = 8

NT = T // P      # 8 query-row tiles
NKC = C // P     # 8 contraction chunks over C
NJ = TC // P     # 16 context chunks
NM = CL // P     # 4 local d-chunks

# Schraudolph fast-exp into bf16 bit pattern via int16:
#   i16 = trunc(x * EXP_A + EXP_B); bf16 = bits(i16)
# max rel err ~3.3% over x in [-10, 8]; scores*scale stay well inside.
EXP_A = float(np.float32(128.0 / np.log(2.0)))
EXP_B = float(np.float32(16256.0 - 5.1))

last_exec_time_ns = None


def _build_nc():
    nc = bacc.Bacc(None, target_bir_lowering=False, debug=False)

    x_d = nc.dram_tensor("x", [NT, P, C], BF, kind="ExternalInput")
    ctxT_d = nc.dram_tensor("ctxT", [P, NKC, TC], F8, kind="ExternalInput")
    wqT_d = nc.dram_tensor("wqT", [P, NKC, CL], F8, kind="ExternalInput")
    wkT_d = nc.dram_tensor("wkT", [P, NKC, CL], F8, kind="ExternalInput")
    wvT_d = nc.dram_tensor("wvT", [P, NKC, CL], F8, kind="ExternalInput")
    woT_d = nc.dram_tensor("woT", [P, NM, C], F8, kind="ExternalInput")
    cb_d = nc.dram_tensor("cb", [P, 9], FP, kind="ExternalInput")
    sel_d = nc.dram_tensor("sel", [P, NM, P], BF, kind="ExternalInput")
    part_d = nc.dram_tensor("partial", [C // P, P, T], BF, kind="ExternalOutput")

    with tile.TileContext(nc) as tc:
        with (
            tc.tile_pool(name="const", bufs=1) as cpool,
            tc.tile_pool(name="acts", bufs=1) as apool,
        ):
            ident_f = cpool.tile([P, P], FP)
            make_identity(nc, ident_f[:])
            ident = cpool.tile([P, P], BF)
            nc.vector.tensor_copy(ident[:], ident_f[:])
            cb = cpool.tile([P, 9], FP)
            nc.sync.dma_start(cb[:], cb_d[:])
            sel = cpool.tile([P, NM, P], BF)
            nc.sync.dma_start(sel[:], sel_d[:])
            eps = cpool.tile([P, 1], FP)
            nc.vector.memset(eps[:], LN_EPS)
            nb4 = cpool.tile([P, 1], FP)
            nc.vector.memset(nb4[:], -4.0)

            wk = apool.tile([P, NKC, CL], F8, tag="wk")
            wv = apool.tile([P, NKC, CL], F8, tag="wv")
            wo = apool.tile([P, NM, C], F8, tag="wo")
            nc.sync.dma_start(wk[:], wkT_d[:])
            nc.sync.dma_start(wv[:], wvT_d[:])
            for k2 in range(NM):
                nc.sync.dma_start(wo[:, k2, :], woT_d[:, k2, :])
            rnT = apool.tile([P, NKC, T], F8, tag="rnT")
            # Scores: stationary kT holds both heads' rows; the moving side is
            # zero-padded per head (qTa rows 64-127 zero, qTb rows 0-63 zero)
            # so every scores matmul stays full-K 128x128 mode.
            qTa = apool.tile([P, NM, T], BF, tag="qTa")
            qTb = apool.tile([P, NM, T], BF, tag="qTb")
            kT = apool.tile([P, NM, TC], BF, tag="kT")
            NJP = NJ // 2
            va8 = apool.tile([P, NJP, 2, NM, 80], F8, tag="va8")
            vvb = apool.tile([P, NJ, NM * (HD + 1)], F8, tag="vvb")
            attnU = apool.tile([P, NM, T], BF, tag="attnU")
            attnT = apool.tile([P, NM, T], F8, tag="attnT")
            # denominator rows live at 32-aligned partitions (BIR requires
            # engine APs to start on partition multiples of 32):
            # row(i, hh) -> partition 32*(2*(i%2)+hh), column half i//2
            denp = apool.tile([P, 2 * T], FP, tag="denp")
            recipp = apool.tile([P, 2 * T], FP, tag="recipp")
            recipb = apool.tile([P, 2 * T], BF, tag="recipb")

            # one-time zero/one fills (DVE, overlapped with initial DMAs)
            nc.gpsimd.memset(qTa[64:128, :, :], 0.0)
            nc.gpsimd.memset(qTb[0:64, :, :], 0.0)
            nc.gpsimd.memset(denp[:], 1.0)
            # ones columns of v' (denominator rows)
            nc.vector.memset(va8[:, :, :, :, HD : HD + 1], 1.0)
            nc.vector.memset(
                vvb[:].rearrange("p j (h c) -> p (j h) c", c=HD + 1)[:, :, HD : HD + 1],
                1.0,
            )
            # preload the Exp activation table before the attention phase
            dummy = cpool.tile([P, 1], BF)

            with (
                tc.tile_pool(name="psmm", bufs=2, space="PSUM") as psmm,
                tc.tile_pool(name="ctx", bufs=1) as ctxpool,
            ):
                # ---- DMAs first; k/v projections lead while the LN
                # chains run on DVE/ACT underneath; transposes + q follow ----
                with (
                    tc.tile_pool(name="xrn", bufs=10) as xpool,
                    tc.tile_pool(name="xst", bufs=6) as spool,
                    tc.tile_pool(name="sqp", bufs=2) as sqpool,
                    tc.tile_pool(name="wqp", bufs=1) as wqpool,
                    tc.tile_pool(name="pstr", bufs=2, space="PSUM") as pstr,
                ):
                    xts = {}
                    for t in range(NT):
                        xts[t] = xpool.tile([P, C], BF, tag="xt", name=f"xt{t}")
                        nc.sync.dma_start(xts[t][:, 0:512], x_d[t][:, 0:512])
                        nc.sync.dma_start(xts[t][:, 512:1024], x_d[t][:, 512:1024])
                    wq = wqpool.tile([P, NKC, CL], F8, tag="wq")
                    nc.sync.dma_start(wq[:], wqT_d[:])
                    ctx8 = ctxpool.tile([P, NKC, TC], F8, tag="ctxT")
                    for k in range(NKC):
                        for hf in range(2):
                            nc.sync.dma_start(
                                ctx8[:, k, hf * 1024 : (hf + 1) * 1024],
                                ctxT_d[:, k, hf * 1024 : (hf + 1) * 1024],
                            )

                    rns = {}

                    def ln_chain(t):
                        xt = xts[t]
                        nmu = spool.tile([P, 1], FP, tag="nmu")
                        nc.vector.reduce_sum(nmu[:], xt[:], axis=mybir.AxisListType.X)
                        nc.scalar.mul(nmu[:], nmu[:], -1.0 / C)
                        sq = sqpool.tile([P, C], BF, tag="sq")
                        ex2 = spool.tile([P, 1], FP, tag="ex2")
                        nc.scalar.activation(
                            sq[:], xt[:], mybir.ActivationFunctionType.Square,
                            accum_out=ex2[:],
                        )
                        var = spool.tile([P, 1], FP, tag="var")
                        nc.scalar.mul(ex2[:], ex2[:], 1.0 / C)
                        mu2 = spool.tile([P, 1], FP, tag="mu2")
                        nc.vector.tensor_mul(mu2[:], nmu[:], nmu[:])
                        nc.vector.tensor_sub(var[:], ex2[:], mu2[:])
                        std = spool.tile([P, 1], FP, tag="std")
                        nc.scalar.activation(
                            std[:], var[:], mybir.ActivationFunctionType.Sqrt,
                            bias=eps[:],
                        )
                        inv = spool.tile([P, 1], FP, tag="inv")
                        nc.vector.reciprocal(inv[:], std[:])
                        rn = xpool.tile([P, C], BF, tag="rn", name=f"rn{t}")
                        nc.vector.scalar_tensor_tensor(
                            out=rn[:], in0=xt[:], scalar=nmu[:],
                            in1=inv[:].to_broadcast((P, C)),
                            op0=mybir.AluOpType.add, op1=mybir.AluOpType.mult,
                        )
                        rns[t] = rn

                    DR = mybir.MatmulPerfMode.DoubleRow
                    ln_t = 0
                    for ch in range(2):
                        hs = slice(ch * (TC // 2), (ch + 1) * (TC // 2))
                        # k projection for this context half (fp8 DoubleRow)
                        for m in range(NM):
                            ps = psmm.tile([P, TC // 2], FP, tag="mm")
                            for n in range(2):
                                ns = slice(ch * 1024 + n * 512, ch * 1024 + (n + 1) * 512)
                                for k in range(NKC // 2):
                                    nc.tensor.matmul(
                                        ps[:, n * 512 : (n + 1) * 512],
                                        wk[:, 2 * k : 2 * k + 2, m * P : (m + 1) * P],
                                        ctx8[:, 2 * k : 2 * k + 2, ns],
                                        start=(k == 0), stop=(k == NKC // 2 - 1),
                                        perf_mode=DR,
                                    )
                            nc.vector.tensor_scalar_add(
                                kT[:, m, hs], ps[:], cb[:, 4 + m : 5 + m],
                            )
                            if ln_t < NT:
                                ln_chain(ln_t)
                                ln_t += 1
                        # v projection for this context half (fp8 DoubleRow)
                        for jj in range(NJ // 2):
                            j = ch * (NJ // 2) + jj
                            ps = psmm.tile([P, CL], FP, tag="mmv")
                            for k in range(NKC // 2):
                                nc.tensor.matmul(
                                    ps[:],
                                    ctx8[:, 2 * k : 2 * k + 2, j * P : (j + 1) * P],
                                    wv[:, 2 * k : 2 * k + 2, :],
                                    start=(k == 0), stop=(k == NKC // 2 - 1),
                                    perf_mode=DR,
                                )
                            psh = ps[:].rearrange("p (h c) -> p h c", c=HD)
                            nc.vector.tensor_copy(
                                va8[:, j // 2, j % 2, :, 0:HD], psh[:, 0::2, :],
                            )
                            nc.vector.tensor_copy(
                                vvb[:, j, :].rearrange(
                                    "p (h c) -> p h c", c=HD + 1)[:, :, 0:HD],
                                psh[:, 1::2, :],
                            )
                            if ln_t < NT:
                                ln_chain(ln_t)
                                ln_t += 1

                    # ---- transposes (PE) after the k/v matmul stream ----
                    for t in range(NT):
                        rn = rns[t]
                        for c4 in range(2):
                            pt = pstr.tile([P, 512], BF, tag="ptr")
                            for cc in range(4):
                                c = 4 * c4 + cc
                                nc.tensor.transpose(
                                    pt[:, cc * P : (cc + 1) * P],
                                    rn[:, c * P : (c + 1) * P],
                                    ident[:],
                                )
                            nc.scalar.copy(
                                rnT[:, 4 * c4 : 4 * c4 + 4, t * P : (t + 1) * P],
                                pt[:].rearrange("p (c q) -> p c q", q=P),
                            )

                    # ---- q projection (fp8 DoubleRow), zero-padded halves ----
                    for m in range(NM):
                        ps = psmm.tile([P, T], FP, tag="mm")
                        for n in range(2):
                            ns = slice(n * 512, (n + 1) * 512)
                            for k in range(NKC // 2):
                                nc.tensor.matmul(
                                    ps[:, ns],
                                    wq[:, 2 * k : 2 * k + 2, m * P : (m + 1) * P],
                                    rnT[:, 2 * k : 2 * k + 2, ns],
                                    start=(k == 0), stop=(k == NKC // 2 - 1),
                                    perf_mode=DR,
                                )
                        nc.vector.tensor_scalar(
                            out=qTa[0:64, m, :], in0=ps[0:64, :],
                            scalar1=cb[0:64, m : m + 1], scalar2=cb[0:64, 8:9],
                            op0=mybir.AluOpType.add, op1=mybir.AluOpType.mult,
                        )
                        nc.vector.tensor_scalar(
                            out=qTb[64:128, m, :], in0=ps[64:128, :],
                            scalar1=cb[64:128, m : m + 1], scalar2=cb[64:128, 8:9],
                            op0=mybir.AluOpType.add, op1=mybir.AluOpType.mult,
                        )
                    # preload exp table after the last Sqrt (input depends
                    # on qTa so the scheduler cannot hoist it before the LN)
                    nc.scalar.activation(
                        dummy[:], qTa[:, 0, 0:1], mybir.ActivationFunctionType.Exp,
                    )

            # ---- attention: scores -> exp (ACT/DVE split) -> attn-out ----
            with (
                tc.tile_pool(name="expa", bufs=2) as eapool,
                tc.tile_pool(name="expb", bufs=2) as ebpool,
            ):
                with (
                    tc.tile_pool(name="pssc", bufs=4, space="PSUM") as pssc,
                    tc.tile_pool(name="psat", bufs=2, space="PSUM") as psat,
                ):
                    for i in range(NM):
                        ph = {}
                        for hh in range(2):
                            ph[hh] = psat.tile(
                                [HD + 1, T], FP, tag="ph", name=f"ph_{i}_{hh}",
                            )
                        DRA = mybir.MatmulPerfMode.DoubleRow
                        b_pending = []
                        a_pending = []

                        def emit_b(lag):
                            jp, pb = b_pending.pop(0)
                            vsb = vvb[:, jp, :].rearrange(
                                "p (h c) -> p h c", c=HD + 1)
                            for n in range(2):
                                ns = slice(n * 512, (n + 1) * 512)
                                nc.tensor.matmul(
                                    ph[1][:, ns], vsb[:, i, :], pb[:, ns],
                                    start=(jp == 0), stop=(jp == NJ - 1),
                                )

                        def emit_a():
                            jpr, pe8 = a_pending.pop(0)
                            for n in range(2):
                                ns = slice(n * 512, (n + 1) * 512)
                                nc.tensor.matmul(
                                    ph[0][:, ns],
                                    va8[:, jpr, :, i, 0 : HD + 1],
                                    pe8[:, :, ns],
                                    start=(jpr == 0), stop=(jpr == NJP - 1),
                                    perf_mode=DRA,
                                )

                        for j in range(NJ):
                            js = slice(j * P, (j + 1) * P)
                            if j % 2 == 0:
                                eta = eapool.tile([P, 2, T], F8, tag="eta",
                                                  name=f"ea{i}_{j // 2}")
                            et_b = ebpool.tile([P, T], I16, tag="etb", name=f"eb{i}_{j}")
                            for n in range(2):
                                ns = slice(n * 512, (n + 1) * 512)
                                psc_a = pssc.tile([P, 512], FP, tag="sc",
                                                  name=f"sa{i}_{j}_{n}")
                                psc_b = pssc.tile([P, 512], FP, tag="sc",
                                                  name=f"sb{i}_{j}_{n}")
                                nc.tensor.matmul(psc_a[:], kT[:, i, js],
                                                 qTa[:, i, ns],
                                                 start=True, stop=True)
                                nc.tensor.matmul(psc_b[:], kT[:, i, js],
                                                 qTb[:, i, ns],
                                                 start=True, stop=True)
                                nc.scalar.activation(
                                    eta[:, j % 2, ns], psc_a[:],
                                    mybir.ActivationFunctionType.Exp,
                                    bias=nb4[:],
                                )
                                nc.vector.tensor_scalar(
                                    out=et_b[:, ns], in0=psc_b[:],
                                    scalar1=EXP_A, scalar2=EXP_B,
                                    op0=mybir.AluOpType.mult, op1=mybir.AluOpType.add,
                                )
                            b_pending.append((j, et_b[:].bitcast(BF)))
                            if j % 2 == 1:
                                a_pending.append((j // 2, eta[:]))
                            if j >= 1:
                                emit_b(1)
                            if j >= 3 and j % 2 == 1:
                                emit_a()
                        emit_b(0)
                        emit_a()
                        # evacuate: data rows -> attnU (ACT), denom rows -> denp (DVE)
                        for hh in range(2):
                            nc.scalar.copy(
                                attnU[64 * hh : 64 * hh + 64, i, :], ph[hh][0:64, :],
                            )
                            dp = 32 * (2 * (i % 2) + hh)
                            dc = (i // 2) * T
                            nc.vector.tensor_copy(
                                denp[dp : dp + 1, dc : dc + T], ph[hh][64:65, :],
                            )
                        if i == 1:
                            nc.vector.reciprocal_approx_fast(
                                recipp[:, 0:T], denp[:, 0:T],
                            )
                            nc.vector.tensor_copy(recipb[:, 0:T], recipp[:, 0:T])

                # ---- deferred normalize ----
                with tc.tile_pool(name="psel", bufs=2, space="PSUM") as psel:
                    nc.vector.reciprocal_approx_fast(
                        recipp[:, T : 2 * T], denp[:, T : 2 * T],
                    )
                    nc.vector.tensor_copy(recipb[:, T : 2 * T], recipp[:, T : 2 * T])
                    for i in range(NM):
                        dc = (i // 2) * T
                        rb = psel.tile([P, T], FP, tag="rb")
                        for n in range(2):
                            ns = slice(n * 512, (n + 1) * 512)
                            nc.tensor.matmul(
                                rb[:, ns], sel[:, i, :],
                                recipb[:, dc + n * 512 : dc + (n + 1) * 512],
                                start=True, stop=True,
                            )
                        nc.vector.tensor_mul(
                            attnT[:, i, :], attnU[:, i, :], rb[:],
                        )

                # ---- out-proj partials ----
                with tc.tile_pool(name="psoc", bufs=3, space="PSUM") as psoc:
                    with tc.tile_pool(name="oev", bufs=3) as opool:
                        for m in range(C // P):
                            po = psoc.tile([P, T], FP, tag="oc")
                            DRO = mybir.MatmulPerfMode.DoubleRow
                            for n in range(2):
                                ns = slice(n * 512, (n + 1) * 512)
                                for k2 in range(NM // 2):
                                    nc.tensor.matmul(
                                        po[:, ns],
                                        wo[:, 2 * k2 : 2 * k2 + 2, m * P : (m + 1) * P],
                                        attnT[:, 2 * k2 : 2 * k2 + 2, ns],
                                        start=(k2 == 0), stop=(k2 == NM // 2 - 1),
                                        perf_mode=DRO,
                                    )
                            ot = opool.tile([P, T], BF, tag="ot")
                            for n in range(2):
                                ns = slice(n * 512, (n + 1) * 512)
                                nc.scalar.copy(ot[:, ns], po[:, ns])
                                nc.sync.dma_start(part_d[m][:, ns], ot[:, ns])

    nc.finalize()
    return nc


_NC_CACHE = {}


def _get_nc():
    if "nc" not in _NC_CACHE:
        _NC_CACHE["nc"] = _build_nc()
    return _NC_CACHE["nc"]


def _quant(w):
    g = np.float32(np.mean(np.abs(w), dtype=np.float64))
    t = np.clip(np.rint(w / (g + np.float32(Q_EPS))), -1.0, 1.0).astype(np.float32)
    return t, g


def _pack_kp(a):
    # [K, M] -> [P, K//P, M] (partition-major chunks)
    k, m = a.shape
    return np.ascontiguousarray(a.reshape(k // P, P, m).transpose(1, 0, 2))


def _bf(a):
    return np.ascontiguousarray(a.astype(ml_dtypes.bfloat16))


def _f8(a):
    return np.ascontiguousarray(a.astype(ml_dtypes.float8_e4m3))


def kernel(**inputs):
    global last_exec_time_ns
    x = np.asarray(inputs["x"], dtype=np.float32)
    ctx = np.asarray(inputs["context"], dtype=np.float32)
    Wq = np.asarray(inputs["Wq"], dtype=np.float32)
    Wk = np.asarray(inputs["Wk"], dtype=np.float32)
    Wv = np.asarray(inputs["Wv"], dtype=np.float32)
    Wo = np.asarray(inputs["Wo"], dtype=np.float32)
    bq = np.asarray(inputs["bq"], dtype=np.float32)
    bk = np.asarray(inputs["bk"], dtype=np.float32)
    bv = np.asarray(inputs["bv"], dtype=np.float32)
    bo = np.asarray(inputs["bo"], dtype=np.float32)
    g_ln = np.asarray(inputs["ln_gamma"], dtype=np.float32)
    b_ln = np.asarray(inputs["ln_beta"], dtype=np.float32)

    Tq, gq = _quant(Wq)
    Tk, gk = _quant(Wk)
    Tv, gv = _quant(Wv)
    To, go = _quant(Wo)

    qb_full = (bq + b_ln @ (gq * Tq).T) / gq          # [C]
    scale = np.float32(gq * gk * SCALE)
    host_bias = bo + bv @ (go * To).T                 # [C]

    # select matrices for the denominator broadcast: recipp partition
    # 32*(2*(i%2)+hh) feeds partitions [64*hh, 64*hh+64) of attnT chunk i
    selm = np.zeros((P, NM, P), dtype=np.float32)
    for i in range(NM):
        selm[32 * (2 * (i % 2)), i, 0:64] = 1.0
        selm[32 * (2 * (i % 2) + 1), i, 64:128] = 1.0

    in_maps = []
    for core in range(NCORES):
        b = core // 2
        g = core % 2
        rows = slice(CL * g, CL * (g + 1))
        wqT = _pack_kp((Tq[rows] * g_ln[None, :]).T)  # [P, 8, 512]
        wkT = _pack_kp(Tk[rows].T)
        wvT = _pack_kp(Tv[rows].T)
        woT = _pack_kp(To[:, rows].T)                 # [P, 4, 1024] ternary
        cbm = np.zeros((P, 9), dtype=np.float32)
        cbm[:, 0:4] = qb_full[rows].reshape(4, P).T
        cbm[:, 4:8] = (bk[rows] / gk).reshape(4, P).T
        cbm[:, 8] = scale
        in_maps.append({
            "x": _bf(x[b].reshape(T // P, P, C)),
            "ctxT": _f8(_pack_kp(np.ascontiguousarray(ctx[b].T))),
            "wqT": _f8(wqT), "wkT": _f8(wkT), "wvT": _f8(wvT), "woT": _f8(woT),
            "cb": cbm,
            "sel": _bf(selm),
        })

    nc = _get_nc()
    trace = os.environ.get("KERNEL_TRACE", "0") == "1"
    res = run_bass_kernel_spmd(nc, in_maps, list(range(NCORES)), trace=trace)
    last_exec_time_ns = res.exec_time_ns

    ogv = np.float32(go * gv)
    out = np.empty((B, T, C), dtype=np.float32)
    for b in range(B):
        p0 = res.results[2 * b]["partial"].astype(np.float32).reshape(C, T)
        p1 = res.results[2 * b + 1]["partial"].astype(np.float32).reshape(C, T)
        out[b] = x[b] + (p0.T + p1.T) * ogv + host_bias[None, :]
    return out


# revision 25
# speedup vs baseline: 1.3475x; 1.0158x over previous
"""Cross-modal attention block on 8 Trainium2 NeuronCores.

Sharding: core = 2*b + g  ->  batch b (4-way data parallel) x head-group g
(2-way tensor parallel over 16 heads -> 8 heads/core).  Each core:
  rownorm(x[b]) -> PE transpose -> q projection (ternary weights, gamma/beta
  folded) ; kT/v projections from pre-transposed context ; per-head
  scoresT = k~^T q~ ; exp split between ScalarE (exact) and VectorE
  (Schraudolph bit-trick into bf16) ; unnormalized attn-out with an appended
  ones-row producing softmax denominators in the same matmul ; deferred
  batch normalize (reciprocal_approx_fast + select-matmul broadcast) ;
  out-proj partial.  Host sums the two partials per batch + residual +
  folded biases.

All matmuls are full 128x128-mode bf16 (scores use zero-padded K so the PE
never enters a tiled mode, which measures as HAM-throttled 1.2 GHz).
"""

import os

import ml_dtypes
import numpy as np

import concourse.bass as bass
import concourse.mybir as mybir
import concourse.tile as tile
from concourse import bacc
from concourse.bass_utils import run_bass_kernel_spmd
from concourse.masks import make_identity

FP = mybir.dt.float32
FPR = mybir.dt.float32r
BF = mybir.dt.bfloat16
I16 = mybir.dt.int16
U8 = mybir.dt.uint8
F8 = mybir.dt.float8e4

B, T, TC, C = 4, 1024, 2048, 1024
H, HD = 16, 64
HL = 8           # heads per core
CL = HL * HD     # 512 local channels
SCALE = HD ** -0.5
LN_EPS = 1e-5
Q_EPS = 1e-5
P = 128
NCORES = 8

NT = T // P      # 8 query-row tiles
NKC = C // P     # 8 contraction chunks over C
NJ = TC // P     # 16 context chunks
NM = CL // P     # 4 local d-chunks

# Schraudolph fast-exp into bf16 bit pattern via int16:
#   i16 = trunc(x * EXP_A + EXP_B); bf16 = bits(i16)
# max rel err ~3.3% over x in [-10, 8]; scores*scale stay well inside.
EXP_A = float(np.float32(128.0 / np.log(2.0)))
EXP_B = float(np.float32(16256.0 - 5.1))
# fp8(e4m3) variant with exp(s-4) shift, relying on saturating f32->u8 store
EXP_A8 = float(np.float32(8.0 / np.log(2.0)))
EXP_B8 = 9.95375930786

last_exec_time_ns = None


def _build_nc():
    nc = bacc.Bacc(None, target_bir_lowering=False, debug=False)

    x_d = nc.dram_tensor("x", [NT, P, C], BF, kind="ExternalInput")
    ctxT_d = nc.dram_tensor("ctxT", [P, NKC, TC], F8, kind="ExternalInput")
    wqT_d = nc.dram_tensor("wqT", [P, NKC, CL], F8, kind="ExternalInput")
    wkT_d = nc.dram_tensor("wkT", [P, NKC, CL], F8, kind="ExternalInput")
    wvT_d = nc.dram_tensor("wvT", [P, NKC, CL], F8, kind="ExternalInput")
    woT_d = nc.dram_tensor("woT", [P, NM, C], F8, kind="ExternalInput")
    cb_d = nc.dram_tensor("cb", [P, 9], FP, kind="ExternalInput")
    sel_d = nc.dram_tensor("sel", [P, NM, P], BF, kind="ExternalInput")
    part_d = nc.dram_tensor("partial", [C // P, P, T], BF, kind="ExternalOutput")

    with tile.TileContext(nc) as tc:
        with (
            tc.tile_pool(name="const", bufs=1) as cpool,
            tc.tile_pool(name="acts", bufs=1) as apool,
        ):
            ident_f = cpool.tile([P, P], FP)
            make_identity(nc, ident_f[:])
            ident = cpool.tile([P, P], BF)
            nc.vector.tensor_copy(ident[:], ident_f[:])
            cb = cpool.tile([P, 9], FP)
            nc.sync.dma_start(cb[:], cb_d[:])
            sel = cpool.tile([P, NM, P], BF)
            nc.sync.dma_start(sel[:], sel_d[:])
            eps = cpool.tile([P, 1], FP)
            nc.vector.memset(eps[:], LN_EPS)
            nb4 = cpool.tile([P, 1], FP)
            nc.vector.memset(nb4[:], -4.0)

            wk = apool.tile([P, NKC, CL], F8, tag="wk")
            wv = apool.tile([P, NKC, CL], F8, tag="wv")
            wo = apool.tile([P, NM, C], F8, tag="wo")
            nc.sync.dma_start(wk[:], wkT_d[:])
            nc.sync.dma_start(wv[:], wvT_d[:])
            for k2 in range(NM):
                nc.sync.dma_start(wo[:, k2, :], woT_d[:, k2, :])
            rnT = apool.tile([P, NKC, T], F8, tag="rnT")
            # Scores: stationary kT holds both heads' rows; the moving side is
            # zero-padded per head (qTa rows 64-127 zero, qTb rows 0-63 zero)
            # so every scores matmul stays full-K 128x128 mode.
            qTa = apool.tile([P, NM, T], BF, tag="qTa")
            qTb = apool.tile([P, NM, T], BF, tag="qTb")
            kT = apool.tile([P, NM, TC], BF, tag="kT")
            NJP = NJ // 2
            va8 = apool.tile([P, NJP, 2, NM, 80], F8, tag="va8")
            vb8 = apool.tile([P, NJP, 2, NM, 80], F8, tag="vb8")
            attnU = apool.tile([P, NM, T], BF, tag="attnU")
            attnT = apool.tile([P, NM, T], F8, tag="attnT")
            # denominator rows live at 32-aligned partitions (BIR requires
            # engine APs to start on partition multiples of 32):
            # row(i, hh) -> partition 32*(2*(i%2)+hh), column half i//2
            denp = apool.tile([P, 2 * T], FP, tag="denp")
            recipp = apool.tile([P, 2 * T], FP, tag="recipp")
            recipb = apool.tile([P, 2 * T], BF, tag="recipb")

            # one-time zero/one fills (DVE, overlapped with initial DMAs)
            nc.gpsimd.memset(qTa[64:128, :, :], 0.0)
            nc.gpsimd.memset(qTb[0:64, :, :], 0.0)
            nc.gpsimd.memset(denp[:], 1.0)
            # ones columns of v' (denominator rows)
            nc.vector.memset(va8[:, :, :, :, HD : HD + 1], 1.0)
            nc.vector.memset(vb8[:, :, :, :, HD : HD + 1], 1.0)
            # preload the Exp activation table before the attention phase
            dummy = cpool.tile([P, 1], BF)

            with (
                tc.tile_pool(name="psmm", bufs=2, space="PSUM") as psmm,
                tc.tile_pool(name="ctx", bufs=1) as ctxpool,
            ):
                # ---- DMAs first; k/v projections lead while the LN
                # chains run on DVE/ACT underneath; transposes + q follow ----
                with (
                    tc.tile_pool(name="xrn", bufs=10) as xpool,
                    tc.tile_pool(name="xst", bufs=6) as spool,
                    tc.tile_pool(name="sqp", bufs=2) as sqpool,
                    tc.tile_pool(name="wqp", bufs=1) as wqpool,
                    tc.tile_pool(name="pstr", bufs=2, space="PSUM") as pstr,
                ):
                    xts = {}
                    for t in range(NT):
                        xts[t] = xpool.tile([P, C], BF, tag="xt", name=f"xt{t}")
                        nc.sync.dma_start(xts[t][:, 0:512], x_d[t][:, 0:512])
                        nc.sync.dma_start(xts[t][:, 512:1024], x_d[t][:, 512:1024])
                    wq = wqpool.tile([P, NKC, CL], F8, tag="wq")
                    nc.sync.dma_start(wq[:], wqT_d[:])
                    ctx8 = ctxpool.tile([P, NKC, TC], F8, tag="ctxT")
                    for k in range(NKC):
                        for hf in range(2):
                            nc.sync.dma_start(
                                ctx8[:, k, hf * 1024 : (hf + 1) * 1024],
                                ctxT_d[:, k, hf * 1024 : (hf + 1) * 1024],
                            )

                    rns = {}

                    def ln_chain(t):
                        xt = xts[t]
                        nmu = spool.tile([P, 1], FP, tag="nmu")
                        nc.vector.reduce_sum(nmu[:], xt[:], axis=mybir.AxisListType.X)
                        nc.scalar.mul(nmu[:], nmu[:], -1.0 / C)
                        sq = sqpool.tile([P, C], BF, tag="sq")
                        ex2 = spool.tile([P, 1], FP, tag="ex2")
                        nc.scalar.activation(
                            sq[:], xt[:], mybir.ActivationFunctionType.Square,
                            accum_out=ex2[:],
                        )
                        var = spool.tile([P, 1], FP, tag="var")
                        nc.scalar.mul(ex2[:], ex2[:], 1.0 / C)
                        mu2 = spool.tile([P, 1], FP, tag="mu2")
                        nc.vector.tensor_mul(mu2[:], nmu[:], nmu[:])
                        nc.vector.tensor_sub(var[:], ex2[:], mu2[:])
                        std = spool.tile([P, 1], FP, tag="std")
                        nc.scalar.activation(
                            std[:], var[:], mybir.ActivationFunctionType.Sqrt,
                            bias=eps[:],
                        )
                        inv = spool.tile([P, 1], FP, tag="inv")
                        nc.vector.reciprocal(inv[:], std[:])
                        rn = xpool.tile([P, C], BF, tag="rn", name=f"rn{t}")
                        nc.vector.scalar_tensor_tensor(
                            out=rn[:], in0=xt[:], scalar=nmu[:],
                            in1=inv[:].to_broadcast((P, C)),
                            op0=mybir.AluOpType.add, op1=mybir.AluOpType.mult,
                        )
                        rns[t] = rn

                    DR = mybir.MatmulPerfMode.DoubleRow
                    ln_t = 0
                    for ch in range(2):
                        hs = slice(ch * (TC // 2), (ch + 1) * (TC // 2))
                        # k projection for this context half (fp8 DoubleRow)
                        for m in range(NM):
                            ps = psmm.tile([P, TC // 2], FP, tag="mm")
                            for n in range(2):
                                ns = slice(ch * 1024 + n * 512, ch * 1024 + (n + 1) * 512)
                                for k in range(NKC // 2):
                                    nc.tensor.matmul(
                                        ps[:, n * 512 : (n + 1) * 512],
                                        wk[:, 2 * k : 2 * k + 2, m * P : (m + 1) * P],
                                        ctx8[:, 2 * k : 2 * k + 2, ns],
                                        start=(k == 0), stop=(k == NKC // 2 - 1),
                                        perf_mode=DR,
                                    )
                            nc.vector.tensor_scalar_add(
                                kT[:, m, hs], ps[:], cb[:, 4 + m : 5 + m],
                            )
                            if ln_t < NT:
                                ln_chain(ln_t)
                                ln_t += 1
                        # v projection for this context half (fp8 DoubleRow)
                        for jj in range(NJ // 2):
                            j = ch * (NJ // 2) + jj
                            ps = psmm.tile([P, CL], FP, tag="mmv")
                            for k in range(NKC // 2):
                                nc.tensor.matmul(
                                    ps[:],
                                    ctx8[:, 2 * k : 2 * k + 2, j * P : (j + 1) * P],
                                    wv[:, 2 * k : 2 * k + 2, :],
                                    start=(k == 0), stop=(k == NKC // 2 - 1),
                                    perf_mode=DR,
                                )
                            psh = ps[:].rearrange("p (h c) -> p h c", c=HD)
                            nc.vector.tensor_copy(
                                va8[:, j // 2, j % 2, :, 0:HD], psh[:, 0::2, :],
                            )
                            nc.vector.tensor_copy(
                                vb8[:, j // 2, j % 2, :, 0:HD], psh[:, 1::2, :],
                            )
                            if ln_t < NT:
                                ln_chain(ln_t)
                                ln_t += 1

                    # ---- transposes (PE) after the k/v matmul stream ----
                    for t in range(NT):
                        rn = rns[t]
                        for c4 in range(2):
                            pt = pstr.tile([P, 512], BF, tag="ptr")
                            for cc in range(4):
                                c = 4 * c4 + cc
                                nc.tensor.transpose(
                                    pt[:, cc * P : (cc + 1) * P],
                                    rn[:, c * P : (c + 1) * P],
                                    ident[:],
                                )
                            nc.scalar.copy(
                                rnT[:, 4 * c4 : 4 * c4 + 4, t * P : (t + 1) * P],
                                pt[:].rearrange("p (c q) -> p c q", q=P),
                            )

                    # ---- q projection (fp8 DoubleRow), zero-padded halves ----
                    for m in range(NM):
                        ps = psmm.tile([P, T], FP, tag="mm")
                        for n in range(2):
                            ns = slice(n * 512, (n + 1) * 512)
                            for k in range(NKC // 2):
                                nc.tensor.matmul(
                                    ps[:, ns],
                                    wq[:, 2 * k : 2 * k + 2, m * P : (m + 1) * P],
                                    rnT[:, 2 * k : 2 * k + 2, ns],
                                    start=(k == 0), stop=(k == NKC // 2 - 1),
                                    perf_mode=DR,
                                )
                        nc.vector.tensor_scalar(
                            out=qTa[0:64, m, :], in0=ps[0:64, :],
                            scalar1=cb[0:64, m : m + 1], scalar2=cb[0:64, 8:9],
                            op0=mybir.AluOpType.add, op1=mybir.AluOpType.mult,
                        )
                        nc.vector.tensor_scalar(
                            out=qTb[64:128, m, :], in0=ps[64:128, :],
                            scalar1=cb[64:128, m : m + 1], scalar2=cb[64:128, 8:9],
                            op0=mybir.AluOpType.add, op1=mybir.AluOpType.mult,
                        )
                    # preload exp table after the last Sqrt (input depends
                    # on qTa so the scheduler cannot hoist it before the LN)
                    nc.scalar.activation(
                        dummy[:], qTa[:, 0, 0:1], mybir.ActivationFunctionType.Exp,
                    )

            # ---- attention: scores -> exp (ACT/DVE split) -> attn-out ----
            with (
                tc.tile_pool(name="expa", bufs=2) as eapool,
                tc.tile_pool(name="expb", bufs=2) as ebpool,
            ):
                with (
                    tc.tile_pool(name="pssc", bufs=4, space="PSUM") as pssc,
                    tc.tile_pool(name="psat", bufs=2, space="PSUM") as psat,
                ):
                    for i in range(NM):
                        ph = {}
                        for hh in range(2):
                            ph[hh] = psat.tile(
                                [HD + 1, T], FP, tag="ph", name=f"ph_{i}_{hh}",
                            )
                        DRA = mybir.MatmulPerfMode.DoubleRow
                        b_pending = []
                        a_pending = []

                        def emit_b(lag):
                            jpr, pb8 = b_pending.pop(0)
                            for n in range(2):
                                ns = slice(n * 512, (n + 1) * 512)
                                nc.tensor.matmul(
                                    ph[1][:, ns],
                                    vb8[:, jpr, :, i, 0 : HD + 1],
                                    pb8[:, :, ns],
                                    start=(jpr == 0), stop=(jpr == NJP - 1),
                                    perf_mode=DRA,
                                )

                        def emit_a():
                            jpr, pe8 = a_pending.pop(0)
                            for n in range(2):
                                ns = slice(n * 512, (n + 1) * 512)
                                nc.tensor.matmul(
                                    ph[0][:, ns],
                                    va8[:, jpr, :, i, 0 : HD + 1],
                                    pe8[:, :, ns],
                                    start=(jpr == 0), stop=(jpr == NJP - 1),
                                    perf_mode=DRA,
                                )

                        for j in range(NJ):
                            js = slice(j * P, (j + 1) * P)
                            if j % 2 == 0:
                                eta = eapool.tile([P, 2, T], F8, tag="eta",
                                                  name=f"ea{i}_{j // 2}")
                                etb = ebpool.tile([P, 2, T], U8, tag="etb",
                                                  name=f"eb{i}_{j // 2}")
                            for n in range(2):
                                ns = slice(n * 512, (n + 1) * 512)
                                psc_a = pssc.tile([P, 512], FP, tag="sc",
                                                  name=f"sa{i}_{j}_{n}")
                                psc_b = pssc.tile([P, 512], FP, tag="sc",
                                                  name=f"sb{i}_{j}_{n}")
                                nc.tensor.matmul(psc_a[:], kT[:, i, js],
                                                 qTa[:, i, ns],
                                                 start=True, stop=True)
                                nc.tensor.matmul(psc_b[:], kT[:, i, js],
                                                 qTb[:, i, ns],
                                                 start=True, stop=True)
                                nc.scalar.activation(
                                    eta[:, j % 2, ns], psc_a[:],
                                    mybir.ActivationFunctionType.Exp,
                                    bias=nb4[:],
                                )
                                nc.vector.tensor_scalar(
                                    out=etb[:, j % 2, ns], in0=psc_b[:],
                                    scalar1=EXP_A8, scalar2=EXP_B8,
                                    op0=mybir.AluOpType.mult, op1=mybir.AluOpType.add,
                                )
                            if j % 2 == 1:
                                a_pending.append((j // 2, eta[:]))
                                b_pending.append((j // 2, etb[:].bitcast(F8)))
                            if j >= 3 and j % 2 == 1:
                                emit_a()
                                emit_b(0)
                        emit_a()
                        emit_b(0)
                        # evacuate: data rows -> attnU (ACT), denom rows -> denp (DVE)
                        for hh in range(2):
                            nc.scalar.copy(
                                attnU[64 * hh : 64 * hh + 64, i, :], ph[hh][0:64, :],
                            )
                            dp = 32 * (2 * (i % 2) + hh)
                            dc = (i // 2) * T
                            nc.vector.tensor_copy(
                                denp[dp : dp + 1, dc : dc + T], ph[hh][64:65, :],
                            )
                        if i == 1:
                            nc.vector.reciprocal_approx_fast(
                                recipp[:, 0:T], denp[:, 0:T],
                            )
                            nc.vector.tensor_copy(recipb[:, 0:T], recipp[:, 0:T])

                # ---- deferred normalize ----
                with tc.tile_pool(name="psel", bufs=2, space="PSUM") as psel:
                    nc.vector.reciprocal_approx_fast(
                        recipp[:, T : 2 * T], denp[:, T : 2 * T],
                    )
                    nc.vector.tensor_copy(recipb[:, T : 2 * T], recipp[:, T : 2 * T])
                    for i in range(NM):
                        dc = (i // 2) * T
                        rb = psel.tile([P, T], FP, tag="rb")
                        for n in range(2):
                            ns = slice(n * 512, (n + 1) * 512)
                            nc.tensor.matmul(
                                rb[:, ns], sel[:, i, :],
                                recipb[:, dc + n * 512 : dc + (n + 1) * 512],
                                start=True, stop=True,
                            )
                        nc.vector.tensor_mul(
                            attnT[:, i, :], attnU[:, i, :], rb[:],
                        )

                # ---- out-proj partials ----
                with tc.tile_pool(name="psoc", bufs=3, space="PSUM") as psoc:
                    with tc.tile_pool(name="oev", bufs=3) as opool:
                        for m in range(C // P):
                            po = psoc.tile([P, T], FP, tag="oc")
                            DRO = mybir.MatmulPerfMode.DoubleRow
                            for n in range(2):
                                ns = slice(n * 512, (n + 1) * 512)
                                for k2 in range(NM // 2):
                                    nc.tensor.matmul(
                                        po[:, ns],
                                        wo[:, 2 * k2 : 2 * k2 + 2, m * P : (m + 1) * P],
                                        attnT[:, 2 * k2 : 2 * k2 + 2, ns],
                                        start=(k2 == 0), stop=(k2 == NM // 2 - 1),
                                        perf_mode=DRO,
                                    )
                            ot = opool.tile([P, T], BF, tag="ot")
                            for n in range(2):
                                ns = slice(n * 512, (n + 1) * 512)
                                nc.scalar.copy(ot[:, ns], po[:, ns])
                                nc.sync.dma_start(part_d[m][:, ns], ot[:, ns])

    nc.finalize()
    return nc


_NC_CACHE = {}


def _get_nc():
    if "nc" not in _NC_CACHE:
        _NC_CACHE["nc"] = _build_nc()
    return _NC_CACHE["nc"]


def _quant(w):
    g = np.float32(np.mean(np.abs(w), dtype=np.float64))
    t = np.clip(np.rint(w / (g + np.float32(Q_EPS))), -1.0, 1.0).astype(np.float32)
    return t, g


def _pack_kp(a):
    # [K, M] -> [P, K//P, M] (partition-major chunks)
    k, m = a.shape
    return np.ascontiguousarray(a.reshape(k // P, P, m).transpose(1, 0, 2))


def _bf(a):
    return np.ascontiguousarray(a.astype(ml_dtypes.bfloat16))


def _f8(a):
    return np.ascontiguousarray(a.astype(ml_dtypes.float8_e4m3))


def kernel(**inputs):
    global last_exec_time_ns
    x = np.asarray(inputs["x"], dtype=np.float32)
    ctx = np.asarray(inputs["context"], dtype=np.float32)
    Wq = np.asarray(inputs["Wq"], dtype=np.float32)
    Wk = np.asarray(inputs["Wk"], dtype=np.float32)
    Wv = np.asarray(inputs["Wv"], dtype=np.float32)
    Wo = np.asarray(inputs["Wo"], dtype=np.float32)
    bq = np.asarray(inputs["bq"], dtype=np.float32)
    bk = np.asarray(inputs["bk"], dtype=np.float32)
    bv = np.asarray(inputs["bv"], dtype=np.float32)
    bo = np.asarray(inputs["bo"], dtype=np.float32)
    g_ln = np.asarray(inputs["ln_gamma"], dtype=np.float32)
    b_ln = np.asarray(inputs["ln_beta"], dtype=np.float32)

    Tq, gq = _quant(Wq)
    Tk, gk = _quant(Wk)
    Tv, gv = _quant(Wv)
    To, go = _quant(Wo)

    qb_full = (bq + b_ln @ (gq * Tq).T) / gq          # [C]
    scale = np.float32(gq * gk * SCALE)
    host_bias = bo + bv @ (go * To).T                 # [C]

    # select matrices for the denominator broadcast: recipp partition
    # 32*(2*(i%2)+hh) feeds partitions [64*hh, 64*hh+64) of attnT chunk i
    selm = np.zeros((P, NM, P), dtype=np.float32)
    for i in range(NM):
        selm[32 * (2 * (i % 2)), i, 0:64] = 1.0
        selm[32 * (2 * (i % 2) + 1), i, 64:128] = 1.0

    in_maps = []
    for core in range(NCORES):
        b = core // 2
        g = core % 2
        rows = slice(CL * g, CL * (g + 1))
        wqT = _pack_kp((Tq[rows] * g_ln[None, :]).T)  # [P, 8, 512]
        wkT = _pack_kp(Tk[rows].T)
        wvT = _pack_kp(Tv[rows].T)
        woT = _pack_kp(To[:, rows].T)                 # [P, 4, 1024] ternary
        cbm = np.zeros((P, 9), dtype=np.float32)
        cbm[:, 0:4] = qb_full[rows].reshape(4, P).T
        cbm[:, 4:8] = (bk[rows] / gk).reshape(4, P).T
        cbm[:, 8] = scale
        in_maps.append({
            "x": _bf(x[b].reshape(T // P, P, C)),
            "ctxT": _f8(_pack_kp(np.ascontiguousarray(ctx[b].T))),
            "wqT": _f8(wqT), "wkT": _f8(wkT), "wvT": _f8(wvT), "woT": _f8(woT),
            "cb": cbm,
            "sel": _bf(selm),
        })

    nc = _get_nc()
    trace = os.environ.get("KERNEL_TRACE", "0") == "1"
    res = run_bass_kernel_spmd(nc, in_maps, list(range(NCORES)), trace=trace)
    last_exec_time_ns = res.exec_time_ns

    ogv = np.float32(go * gv)
    out = np.empty((B, T, C), dtype=np.float32)
    for b in range(B):
        p0 = res.results[2 * b]["partial"].astype(np.float32).reshape(C, T)
        p1 = res.results[2 * b + 1]["partial"].astype(np.float32).reshape(C, T)
        out[b] = x[b] + (p0.T + p1.T) * ogv + host_bias[None, :]
    return out
